# revision 1
# baseline (speedup 1.0000x reference)
"""Trainium2 Bass kernel for a pre-norm transformer block (dense_transformer).

Computation (per reference):
    x = x + Attn(LN1(x));  x = x + MLP(LN2(x))
with causal multi-head attention (H=16 heads, D=64) and a 4E ReLU MLP.

Sharding strategy (no collectives):
    8 cores = 2 batches x 4 query-blocks of 512 tokens.  Each core computes
    the full block output for its 512 query rows.  K/V are recomputed per
    core for the causal prefix.  To keep the SPMD program identical across
    cores, the context is ROTATED so the query block always sits in slots
    [T-512, T): slots [0, pad) are zero padding, masked via a per-partition
    additive bias (-1e9) fused into the softmax exp; the causal diagonal
    band mask is a fixed tensor shared by all cores.

Layouts: activations are kept feature-major (E on partitions, tokens on the
free axis) the whole way through, so no on-device transposes are needed.
The host transposes x / assembles the output.  Matmuls run in bf16 with
fp32 PSUM accumulation; LN / softmax / residuals are fp32.  Softmax row
sums come for free from a ones-column appended to V (M=65 matmuls).
"""

from dataclasses import dataclass

import numpy as np
import ml_dtypes

import concourse.bass as bass  # noqa: F401
import concourse.mybir as mybir
import concourse.tile as tile
from concourse import bacc
from concourse import bass_utils

F32 = mybir.dt.float32
BF16 = mybir.dt.bfloat16
AF = mybir.ActivationFunctionType
OP = mybir.AluOpType
NPBF16 = ml_dtypes.bfloat16

P = 128
NEG = -1.0e9


@dataclass(frozen=True)
class Cfg:
    B: int = 2
    T: int = 2048
    E: int = 1024
    H: int = 16
    D: int = 64
    NC: int = 8
    eps: float = 1e-5

    @property
    def CPB(self):
        return self.NC // self.B

    @property
    def Tq(self):
        return self.T // self.CPB

    @property
    def KE(self):
        return self.E // P

    @property
    def TK(self):
        return self.T // P

    @property
    def HP(self):
        return self.H // 2

    @property
    def NB(self):
        return self.Tq // P

    @property
    def F(self):
        return 4 * self.E

    @property
    def KF(self):
        return self.F // P

    @property
    def TCH(self):
        return min(512, self.T)

    @property
    def NQC(self):
        return self.T // self.TCH

    def check(self):
        assert self.D == 64 and self.E == self.H * self.D
        assert self.Tq <= 512 and self.Tq % P == 0
        assert self.T % self.TCH == 0 and self.E % P == 0 and self.F % P == 0
        assert self.NC % self.B == 0 and self.H % 4 == 0 and self.KE % 2 == 0


CFG = Cfg()


class Pools:
    """Tile pools with explicit open/close (LIFO per side, per space)."""

    def __init__(self, tc, prefix=""):
        self.tc = tc
        self.prefix = prefix
        self.live = {}

    def open(self, key, bufs, space=None, side=None):
        kw = dict(name=self.prefix + key, bufs=bufs)
        if space:
            kw["space"] = space
        if side:
            kw["side"] = side
        cm = self.tc.tile_pool(**kw)
        pool = cm.__enter__()
        self.live[key] = cm
        return pool

    def close(self, *keys):
        for key in keys:
            self.live.pop(key).__exit__(None, None, None)

    def close_all(self):
        for key in reversed(list(self.live)):
            self.close(key)


def _emit(tc, c: Cfg, d, reps: int = 1):
    for _rep in range(reps):
        _emit_one(tc, c, d, _rep)


def _emit_one(tc, c: Cfg, d, rep: int):
    nc = tc.nc
    E, T, Tq, H, D = c.E, c.T, c.Tq, c.H, c.D
    KE, TK, HP, NB, KF = c.KE, c.TK, c.HP, c.NB, c.KF
    TCH, NQC = c.TCH, c.NQC
    DQ = D + 1
    SCL = 1.0 / float(np.sqrt(D))

    pp = Pools(tc, prefix=f"r{rep}_")

    # ---------------- constants (whole-kernel lifetime) --------------------
    const = pp.open("const", 1)
    ones_bf = const.tile([P, 1], BF16, name="ones_bf")
    nc.vector.memset(ones_bf[:], 1.0)
    ones_f1 = const.tile([1, P], F32, name="ones_f1")
    nc.vector.memset(ones_f1[:], 1.0)
    onehot = const.tile([P, HP * P], BF16, name="onehot")
    nc.sync.dma_start(onehot[:], d["onehot"])
    colmask = const.tile([P, TK], F32, name="colmask")
    nc.sync.dma_start(colmask[:], d["colmask"])
    band01 = const.tile([P, NB * 4 * Tq], BF16, name="band01")
    nc.sync.dma_start(band01[:], d["band01"])
    gbt = {}
    for nm, cols in [
        ("ln1g", KE), ("ln1b", KE), ("ln2g", KE), ("ln2b", KE),
        ("boc", KE), ("mb1", KF), ("mb2", KE),
    ]:
        gbt[nm] = const.tile([P, cols], F32, name=nm + "_t")
        nc.sync.dma_start(gbt[nm][:], d[nm])

    # ---------------- long-lived activations ------------------------------
    p_xq = pp.open("xqp", 1)
    xq = [p_xq.tile([P, Tq], F32, name=f"xq{e}") for e in range(KE)]

    # ======================================================================
    # Phase 0: load x^T + LayerNorm1 (feature-major, per-column stats)
    # ======================================================================
    p_xn = pp.open("xnp", 1)
    p_xt = pp.open("xtp", 1)
    p_tmp = pp.open("ln_tmp", 4)
    p_rows = pp.open("ln_rows", 1)
    ps_st = pp.open("ln_st", 1, "PSUM")
    ps_bc = pp.open("ln_bc", 2, "PSUM")

    ps_wm = pp.open("warm_ps", 1, "PSUM")
    wmp = ps_wm.tile([1, TCH], F32, name="wmp")
    for _w in range(24):
        nc.tensor.matmul(
            wmp[:], ones_bf[:], band01[:, 0:TCH], start=True, stop=True
        )
    pp.close("warm_ps")

    xt = [p_xt.tile([P, T], F32, name=f"xt{e}") for e in range(KE)]
    xn = [p_xn.tile([P, T], BF16, name=f"xn{e}") for e in range(KE)]
    for e in range(KE):
        nc.sync.dma_start(xt[e][:], d["xt"][e * P : (e + 1) * P, :])
        nc.vector.tensor_copy(xq[e][:], xt[e][:, T - Tq :])

    for ci in range(NQC):
        cs = slice(ci * TCH, (ci + 1) * TCH)
        s1 = ps_st.tile([1, TCH], F32, name="s1")
        s2 = ps_st.tile([1, TCH], F32, name="s2")
        for e in range(KE):
            xbf = p_tmp.tile([P, TCH], BF16, name="xbf")
            nc.vector.tensor_copy(xbf[:], xt[e][:, cs])
            x2 = p_tmp.tile([P, TCH], BF16, name="x2bf")
            nc.scalar.square(x2[:], xt[e][:, cs])
            nc.tensor.matmul(s1[:], ones_bf[:], xbf[:], start=(e == 0), stop=(e == KE - 1))
            nc.tensor.matmul(s2[:], ones_bf[:], x2[:], start=(e == 0), stop=(e == KE - 1))
        mu = p_rows.tile([1, TCH], F32, name="mu")
        nc.vector.tensor_scalar_mul(mu[:], s1[:], 1.0 / E)
        ve = p_rows.tile([1, TCH], F32, name="ve")
        nc.vector.tensor_scalar(ve[:], s2[:], 1.0 / E, c.eps, OP.mult, OP.add)
        mu2 = p_rows.tile([1, TCH], F32, name="mu2")
        nc.vector.tensor_tensor(mu2[:], mu[:], mu[:], OP.mult)
        vee = p_rows.tile([1, TCH], F32, name="vee")
        nc.vector.tensor_tensor(vee[:], ve[:], mu2[:], OP.subtract)
        lv = p_rows.tile([1, TCH], F32, name="lv")
        nc.scalar.activation(lv[:], vee[:], AF.Ln)
        rstd = p_rows.tile([1, TCH], F32, name="rstd")
        nc.scalar.activation(rstd[:], lv[:], AF.Exp, scale=-0.5)

        mub = ps_bc.tile([P, TCH], F32, name="mub")
        nc.tensor.matmul(mub[:], ones_f1[:], mu[:], start=True, stop=True)
        rsb = ps_bc.tile([P, TCH], F32, name="rsb")
        nc.tensor.matmul(rsb[:], ones_f1[:], rstd[:], start=True, stop=True)

        for e in range(KE):
            t1 = p_tmp.tile([P, TCH], F32, name="t1")
            nc.vector.tensor_tensor(t1[:], xt[e][:, cs], mub[:], OP.subtract)
            t2 = p_tmp.tile([P, TCH], F32, name="t2")
            nc.vector.tensor_tensor(t2[:], t1[:], rsb[:], OP.mult)
            nc.vector.tensor_scalar(
                xn[e][:, cs], t2[:],
                gbt["ln1g"][:, e : e + 1], gbt["ln1b"][:, e : e + 1],
                OP.mult, OP.add,
            )
    pp.close("ln_rows", "ln_tmp", "xtp", "ln_bc", "ln_st")

    # ======================================================================
    # Phase 1: QKV projections
    # ======================================================================
    p_wo = pp.open("wop", 1)
    wo_sb = [p_wo.tile([P, E], BF16, name=f"wo{j}") for j in range(HP)]
    for j in range(HP):
        nc.sync.dma_start(wo_sb[j][:], d["wo"][j * P : (j + 1) * P, :])

    p_qt = pp.open("qtp", 1)
    p_kt = pp.open("ktp", 1)
    p_vs = pp.open("vsp", 1)
    p_wcb = pp.open("wcb", 3)
    p_wv = pp.open("wvp", 1)
    ps_qkv = pp.open("qkv_ps", 2, "PSUM")

    qt = [p_qt.tile([P, Tq], BF16, name=f"qt{j}") for j in range(HP)]
    kt = [p_kt.tile([P, T], BF16, name=f"kt{j}") for j in range(HP)]
    vsb = [p_vs.tile([P, H * D], BF16, name=f"vsb{t}") for t in range(TK)]

    def k_proj(j, psum_pool, nm="k_ps"):
        wk_j = p_wcb.tile([P, KE, P], BF16, name="wkcb")
        nc.sync.dma_start(
            wk_j[:],
            d["wk"].rearrange("(e p) m -> p e m", p=P)[:, :, j * P : (j + 1) * P],
        )
        for ci in range(NQC):
            cs = slice(ci * TCH, (ci + 1) * TCH)
            ps = psum_pool.tile([P, TCH], F32, name=nm)
            for e in range(KE):
                nc.tensor.matmul(
                    ps[:], wk_j[:, e, :], xn[e][:, cs],
                    start=(e == 0), stop=(e == KE - 1),
                )
            nc.vector.tensor_copy(kt[j][:, cs], ps[:])

    def q_proj(j):
        wq_j = p_wcb.tile([P, KE, P], BF16, name="wqcb")
        nc.sync.dma_start(
            wq_j[:],
            d["wq"].rearrange("(e p) m -> p e m", p=P)[:, :, j * P : (j + 1) * P],
        )
        ps = ps_qkv.tile([P, Tq], F32, name="q_ps")
        for e in range(KE):
            nc.tensor.matmul(
                ps[:], wq_j[:, e, :], xn[e][:, T - Tq :],
                start=(e == 0), stop=(e == KE - 1),
            )
        nc.any.tensor_copy(qt[j][:], ps[:])

    # Q/K for the first attention group up front so its score/exp stream can
    # start while the remaining projections run; the other K projections are
    # emitted inside the attention loop.
    for j in range(min(2, HP)):
        q_proj(j)
    for j in range(min(2, HP)):
        k_proj(j, ps_qkv)
    for j in range(2, HP):
        q_proj(j)

    wv_sb = [p_wv.tile([P, E], BF16, name=f"wv{e}") for e in range(KE)]
    for e in range(KE):
        nc.sync.dma_start(wv_sb[e][:], d["wv"][e * P : (e + 1) * P, :])
    ECH = min(512, E)
    NEC = E // ECH

    def v_proj(g, psum_pool, nm="v_ps"):
        gs = slice(g * ECH, (g + 1) * ECH)
        for t in range(TK):
            ps = psum_pool.tile([P, ECH], F32, name=nm)
            for e in range(KE):
                nc.tensor.matmul(
                    ps[:], xn[e][:, t * P : (t + 1) * P], wv_sb[e][:, gs],
                    start=(e == 0), stop=(e == KE - 1),
                )
            nc.vector.tensor_copy(vsb[t][:, gs], ps[:])

    # V columns 0:512 (heads 0-7) feed attention groups 0-1; the second
    # chunk is emitted inside the attention loop to overlap the exp stream.
    v_proj(0, ps_qkv)
    pp.close("qkv_ps")

    # ======================================================================
    # Phase 2: attention (4-head groups; row-paired scores, col-paired attnV;
    # remaining K projections interleaved to keep PE fed under the exp stream)
    # ======================================================================
    HG = 4
    NG = H // HG
    GP = HG // 2

    p_ao = pp.open("aop", 1, side="right")
    p_rs = pp.open("rsp", 1)
    p_pr = pp.open("probs", 2)
    p_st2 = pp.open("rstage", 1)
    ps_k2 = pp.open("k2_ps", 1, "PSUM")
    ps_sc = pp.open("sc_ps", 1, "PSUM")
    ps_o = pp.open("o_ps", 1, "PSUM")
    ps_rs = pp.open("rs_ps", 1, "PSUM")

    aop_t = [p_ao.tile([P, Tq], BF16, name=f"aop{j}") for j in range(HP)]
    rs_all = p_rs.tile([P, Tq], F32, name="rs_all")
    nc.vector.memset(rs_all[:], 1.0)
    lrs = p_rs.tile([P, Tq], F32, name="lrs")
    irs_bf = p_rs.tile([P, Tq], BF16, name="irs_bf")
    nc.vector.memset(irs_bf[:], 0.0)

    for g in range(NG):
        opair = [ps_o.tile([P, Tq], F32, name=f"opair{i}") for i in range(GP)]
        rsps = ps_rs.tile([P, Tq], F32, name="rsps")
        for t in range(TK):
            ss = ps_sc.tile([P, HG * Tq], F32, name="ss")
            for i in range(GP):
                j = g * GP + i
                for s in (0, 1):
                    h01 = 2 * i + s
                    nc.tensor.matmul(
                        ss[:, h01 * Tq : (h01 + 1) * Tq],
                        kt[j][s * 64 : (s + 1) * 64, t * P : (t + 1) * P],
                        qt[j][s * 64 : (s + 1) * 64, :],
                        start=True, stop=True,
                        tile_position=(s * 64, 0),
                    )
            pr = p_pr.tile([P, HG * Tq], BF16, name="pr")
            nc.scalar.activation(
                pr[:], ss[:], AF.Exp, bias=colmask[:, t : t + 1], scale=SCL
            )
            bt = t - (TK - NB)
            if bt >= 0:
                nc.vector.tensor_tensor(
                    pr[:], pr[:],
                    band01[:, bt * HG * Tq : (bt + 1) * HG * Tq], OP.mult,
                )
            for i in range(GP):
                j = g * GP + i
                for s in (0, 1):
                    h = 2 * j + s
                    h01 = 2 * i + s
                    nc.tensor.matmul(
                        opair[i][s * 64 : (s + 1) * 64, :],
                        vsb[t][:, h * D : (h + 1) * D],
                        pr[:, h01 * Tq : (h01 + 1) * Tq],
                        start=(t == 0), stop=(t == TK - 1),
                        tile_position=(0, s * 64),
                        skip_group_check=True,
                    )
            for h01 in range(HG):
                nc.tensor.matmul(
                    rsps[32 * h01 : 32 * h01 + 1, :],
                    ones_bf[:],
                    pr[:, h01 * Tq : (h01 + 1) * Tq],
                    start=(t == 0), stop=(t == TK - 1),
                    tile_position=(0, 32 * h01),
                    skip_group_check=True,
                )
        for i in range(GP):
            nc.vector.tensor_copy(aop_t[g * GP + i][:], opair[i][:])
        st = p_st2.tile([P, Tq], F32, name="rstage")
        for h01 in range(HG):
            nc.vector.tensor_copy(
                st[32 * h01 : 32 * h01 + 1, :], rsps[32 * h01 : 32 * h01 + 1, :]
            )
        for h01 in range(HG):
            nc.sync.dma_start(
                rs_all[32 * g + h01 : 32 * g + h01 + 1, :],
                st[32 * h01 : 32 * h01 + 1, :],
            )
        # emit the next group's K projections here: they fill the tensor
        # engine while this group's exp/attnV pipeline drains
        if g + 1 < NG:
            k_proj(2 * (g + 1), ps_k2, nm="kv_ps")
            k_proj(2 * (g + 1) + 1, ps_k2, nm="kv_ps")
        if g == 1 and NEC > 1:
            v_proj(1, ps_k2, nm="kv_ps")

    pp.close("rstage", "probs")
    pp.close("rs_ps", "o_ps", "sc_ps", "k2_ps")

    # softmax denominators (1/rs via exp(-ln)) -> normalize pairs
    p_nb = pp.open("nrm", 2)
    ps_n = pp.open("n_ps", 2, "PSUM")
    nc.scalar.activation(lrs[:], rs_all[:], AF.Ln)
    nc.scalar.activation(irs_bf[:], lrs[:], AF.Exp, scale=-1.0)
    for j in range(HP):
        bb = 64 * (j // 4)  # lhsT base partition must be in {0, 32, 64}
        nb = ps_n.tile([P, Tq], F32, name="nb")
        nc.tensor.matmul(
            nb[:],
            onehot[bb : bb + 64, j * P : (j + 1) * P],
            irs_bf[bb : bb + 64, :],
            start=True, stop=True,
        )
        nbs = p_nb.tile([P, Tq], BF16, name="nbs")
        nc.vector.tensor_copy(nbs[:], nb[:])
        nc.vector.tensor_tensor(aop_t[j][:], aop_t[j][:], nbs[:], OP.mult)
    pp.close("nrm", "n_ps")
    pp.close("rsp", "wvp", "wcb")
    pp.close("vsp", "ktp", "qtp")

    # ======================================================================
    # Phase 3: out-projection + residual -> xres; LayerNorm2 -> xn2
    # ======================================================================
    p_xr = pp.open("xrp", 1)
    p_x2 = pp.open("xn2p", 1)
    ps_ao = pp.open("ao_ps", 2, "PSUM")

    xres = [p_xr.tile([P, Tq], F32, name=f"xres{e}") for e in range(KE)]
    xn2 = [p_x2.tile([P, Tq], BF16, name=f"xn2{e}") for e in range(KE)]

    for e in range(KE):
        ps = ps_ao.tile([P, Tq], F32, name="aops")
        for j in range(HP):
            nc.tensor.matmul(
                ps[:], wo_sb[j][:, e * P : (e + 1) * P], aop_t[j][:],
                start=(j == 0), stop=(j == HP - 1),
            )
        nc.vector.scalar_tensor_tensor(
            xres[e][:], ps[:], gbt["boc"][:, e : e + 1], xq[e][:], OP.add, OP.add
        )
    pp.close("ao_ps", "aop")

    # LayerNorm2 over the Tq query columns
    p_tmp = pp.open("ln2_tmp", 3)
    p_rows = pp.open("ln2_rows", 1)
    ps_st = pp.open("ln2_st", 1, "PSUM")
    ps_bc = pp.open("ln2_bc", 1, "PSUM")
    s1 = ps_st.tile([1, Tq], F32, name="s1b")
    s2 = ps_st.tile([1, Tq], F32, name="s2b")
    for e in range(KE):
        xbf = p_tmp.tile([P, Tq], BF16, name="xbf2")
        nc.vector.tensor_copy(xbf[:], xres[e][:])
        x2 = p_tmp.tile([P, Tq], BF16, name="x2bf2")
        nc.scalar.square(x2[:], xres[e][:])
        nc.tensor.matmul(s1[:], ones_bf[:], xbf[:], start=(e == 0), stop=(e == KE - 1))
        nc.tensor.matmul(s2[:], ones_bf[:], x2[:], start=(e == 0), stop=(e == KE - 1))
    mu = p_rows.tile([1, Tq], F32, name="mu_2")
    nc.vector.tensor_scalar_mul(mu[:], s1[:], 1.0 / E)
    ve = p_rows.tile([1, Tq], F32, name="ve_2")
    nc.vector.tensor_scalar(ve[:], s2[:], 1.0 / E, c.eps, OP.mult, OP.add)
    mu2 = p_rows.tile([1, Tq], F32, name="mu2_2")
    nc.vector.tensor_tensor(mu2[:], mu[:], mu[:], OP.mult)
    vee = p_rows.tile([1, Tq], F32, name="vee_2")
    nc.vector.tensor_tensor(vee[:], ve[:], mu2[:], OP.subtract)
    lv = p_rows.tile([1, Tq], F32, name="lv_2")
    nc.scalar.activation(lv[:], vee[:], AF.Ln)
    rstd = p_rows.tile([1, Tq], F32, name="rstd_2")
    nc.scalar.activation(rstd[:], lv[:], AF.Exp, scale=-0.5)
    mub = ps_bc.tile([P, Tq], F32, name="mub2")
    nc.tensor.matmul(mub[:], ones_f1[:], mu[:], start=True, stop=True)
    rsb = ps_bc.tile([P, Tq], F32, name="rsb2")
    nc.tensor.matmul(rsb[:], ones_f1[:], rstd[:], start=True, stop=True)
    for e in range(KE):
        t1 = p_tmp.tile([P, Tq], F32, name="t1b")
        nc.vector.tensor_tensor(t1[:], xres[e][:], mub[:], OP.subtract)
        t2 = p_tmp.tile([P, Tq], F32, name="t2b")
        nc.vector.tensor_tensor(t2[:], t1[:], rsb[:], OP.mult)
        nc.vector.tensor_scalar(
            xn2[e][:], t2[:],
            gbt["ln2g"][:, e : e + 1], gbt["ln2b"][:, e : e + 1],
            OP.mult, OP.add,
        )
    pp.close("ln2_rows", "ln2_tmp", "ln2_bc", "ln2_st")

    # ======================================================================
    # Phase 4+5: MLP (layer 1 streamed with first-half layer 2, then rest)
    # ======================================================================
    EH = min(KE, 6)  # h2 chunks accumulated under MLP1 (PSUM: 6 + 2 h1 bufs)
    p_h1 = pp.open("h1p", 1, side="right")
    p_w2 = pp.open("w2s", 3)
    p_out = pp.open("outp", 2)
    p_w1 = pp.open("w1s", 3)
    ps_h1 = pp.open("h1_ps", 2, "PSUM")
    ps_h2a = pp.open("h2a_ps", 1, "PSUM")

    h1 = [p_h1.tile([P, Tq], BF16, name=f"h1{f}") for f in range(KF)]
    h2a = [ps_h2a.tile([P, Tq], F32, name=f"h2a{e}") for e in range(EH)]
    for f in range(KF):
        w1f = p_w1.tile([P, KE, P], BF16, name="w1cb")
        nc.sync.dma_start(
            w1f[:],
            d["w1"].rearrange("(e p) m -> p e m", p=P)[:, :, f * P : (f + 1) * P],
        )
        ps = ps_h1.tile([P, Tq], F32, name="h1ps")
        for e in range(KE):
            nc.tensor.matmul(
                ps[:], w1f[:, e, :], xn2[e][:], start=(e == 0), stop=(e == KE - 1)
            )
        nc.scalar.activation(
            h1[f][:], ps[:], AF.Relu, bias=gbt["mb1"][:, f : f + 1], scale=1.0
        )
        w2f = p_w2.tile([P, E], BF16, name="w2sa")
        nc.sync.dma_start(w2f[:], d["w2"][f * P : (f + 1) * P, :])
        for e in range(EH):
            nc.tensor.matmul(
                h2a[e][:], w2f[:, e * P : (e + 1) * P], h1[f][:],
                start=(f == 0), stop=(f == KF - 1),
            )
    for e in range(EH):
        of = p_out.tile([P, Tq], F32, name="outf")
        nc.vector.scalar_tensor_tensor(
            of[:], h2a[e][:], gbt["mb2"][:, e : e + 1], xres[e][:], OP.add, OP.add
        )
        nc.sync.dma_start(d["out_t"][e * P : (e + 1) * P, :], of[:])
    pp.close("w1s", "h2a_ps", "h1_ps")

    if EH < KE:
        ps_h2b = pp.open("h2b_ps", 1, "PSUM")
        h2b = [ps_h2b.tile([P, Tq], F32, name=f"h2b{e}") for e in range(KE - EH)]
        for f in range(KF):
            w2f = p_w2.tile([P, E], BF16, name="w2sb")
            nc.sync.dma_start(w2f[:], d["w2"][f * P : (f + 1) * P, :])
            for i, e in enumerate(range(EH, KE)):
                nc.tensor.matmul(
                    h2b[i][:], w2f[:, e * P : (e + 1) * P], h1[f][:],
                    start=(f == 0), stop=(f == KF - 1),
                )
        for i, e in enumerate(range(EH, KE)):
            of = p_out.tile([P, Tq], F32, name="outf")
            nc.vector.scalar_tensor_tensor(
                of[:], h2b[i][:], gbt["mb2"][:, e : e + 1], xres[e][:], OP.add, OP.add
            )
            nc.sync.dma_start(d["out_t"][e * P : (e + 1) * P, :], of[:])

    pp.close_all()


def build_program(c: Cfg = CFG, reps: int = 1):
    c.check()
    nc = bacc.Bacc(
        "TRN2",
        target_bir_lowering=False,
        debug=False,
        enable_asserts=False,
        num_devices=c.NC,
    )
    d = {}
    d["xt"] = nc.dram_tensor("xt", [c.E, c.T], F32, kind="ExternalInput").ap()
    d["wq"] = nc.dram_tensor("wq", [c.E, c.E], BF16, kind="ExternalInput").ap()
    d["wk"] = nc.dram_tensor("wk", [c.E, c.E], BF16, kind="ExternalInput").ap()
    d["wv"] = nc.dram_tensor("wv", [c.E, c.E], BF16, kind="ExternalInput").ap()
    d["wo"] = nc.dram_tensor("wo", [c.E, c.E], BF16, kind="ExternalInput").ap()
    d["w1"] = nc.dram_tensor("w1", [c.E, c.F], BF16, kind="ExternalInput").ap()
    d["w2"] = nc.dram_tensor("w2", [c.F, c.E], BF16, kind="ExternalInput").ap()
    for nm, cols in [
        ("ln1g", c.KE), ("ln1b", c.KE), ("ln2g", c.KE), ("ln2b", c.KE),
        ("boc", c.KE), ("mb1", c.KF), ("mb2", c.KE),
    ]:
        d[nm] = nc.dram_tensor(nm, [P, cols], F32, kind="ExternalInput").ap()
    d["colmask"] = nc.dram_tensor("colmask", [P, c.TK], F32, kind="ExternalInput").ap()
    d["onehot"] = nc.dram_tensor(
        "onehot", [128, c.HP * 128], BF16, kind="ExternalInput"
    ).ap()
    d["band01"] = nc.dram_tensor(
        "band01", [P, c.NB * 4 * c.Tq], BF16, kind="ExternalInput"
    ).ap()
    d["out_t"] = nc.dram_tensor("out_t", [c.E, c.Tq], F32, kind="ExternalOutput").ap()

    with tile.TileContext(nc) as tc:
        _emit(tc, c, d, reps=reps)
    nc.compile()
    return nc


# --------------------------------------------------------------------------
# host side
# --------------------------------------------------------------------------
def shard_inputs(inputs, c: Cfg = CFG):
    x = np.ascontiguousarray(np.asarray(inputs["x"], np.float32))
    bf = lambda a: np.ascontiguousarray(np.asarray(a, np.float32)).astype(NPBF16)


    chunks = lambda v, k: np.ascontiguousarray(
        np.asarray(v, np.float32).reshape(k, P).T
    )
    com = {
        "wq": bf(inputs["Wq"]),
        "wk": bf(inputs["Wk"]),
        "wv": bf(inputs["Wv"]),
        "wo": bf(inputs["Wo"]),
        "w1": bf(inputs["W1"]),
        "w2": bf(inputs["W2"]),
        "ln1g": chunks(inputs["ln1_g"], c.KE),
        "ln1b": chunks(inputs["ln1_b"], c.KE),
        "ln2g": chunks(inputs["ln2_g"], c.KE),
        "ln2b": chunks(inputs["ln2_b"], c.KE),
        "boc": chunks(inputs["bo"], c.KE),
        "mb1": chunks(inputs["b1"], c.KF),
        "mb2": chunks(inputs["b2"], c.KE),
    }

    p_idx = np.arange(P)[:, None]
    tq_idx = np.arange(c.Tq)[None, :]
    band = np.zeros((P, c.NB * 4 * c.Tq), np.float32)
    for jb in range(c.NB):
        m = (tq_idx >= (jb * P + p_idx)).astype(np.float32)
        for s in range(4):
            band[:, jb * 4 * c.Tq + s * c.Tq : jb * 4 * c.Tq + (s + 1) * c.Tq] = m
    com["band01"] = band.astype(NPBF16)
    oh = np.zeros((P, c.HP * P), np.float32)
    for j in range(c.HP):
        g, i = j // 2, j % 2
        oh[32 * g + 2 * i, j * P : j * P + 64] = 1.0
        oh[32 * g + 2 * i + 1, j * P + 64 : (j + 1) * P] = 1.0
    com["onehot"] = oh.astype(NPBF16)

    slot = np.arange(c.T)
    maps = []
    for core in range(c.NC):
        b, qi = core // c.CPB, core % c.CPB
        qoff = qi * c.Tq
        pad = c.T - qoff - c.Tq
        ctx = np.zeros((c.T, c.E), np.float32)
        ctx[pad:, :] = x[b, : qoff + c.Tq, :]
        colmask = np.ascontiguousarray(
            np.where(slot.reshape(c.TK, P).T < pad, NEG, 0.0).astype(np.float32)
        )
        m = dict(com)
        m["xt"] = np.ascontiguousarray(ctx.T)
        m["colmask"] = colmask
        maps.append(m)
    return maps


def assemble(results, c: Cfg = CFG):
    out = np.empty((c.B, c.T, c.E), np.float32)
    for core in range(c.NC):
        b, qi = core // c.CPB, core % c.CPB
        out[b, qi * c.Tq : (qi + 1) * c.Tq, :] = results[core]["out_t"].T
    return out


_NC_CACHE = {}


def _get_nc(c: Cfg = CFG):
    if c not in _NC_CACHE:
        _NC_CACHE[c] = build_program(c)
    return _NC_CACHE[c]


LAST_RESULT = None


def kernel(**inputs):
    global LAST_RESULT
    c = CFG
    nc = _get_nc(c)
    maps = shard_inputs(inputs, c)
    res = bass_utils.run_bass_kernel_spmd(nc, maps, core_ids=list(range(c.NC)))
    LAST_RESULT = res
    return assemble(res.results, c)



# revision 8
# speedup vs baseline: 1.1457x; 1.1457x over previous
"""Trainium2 Bass kernel for a pre-norm transformer block (dense_transformer).

Computation (per reference):
    x = x + Attn(LN1(x));  x = x + MLP(LN2(x))
with causal multi-head attention (H=16 heads, D=64) and a 4E ReLU MLP.

Sharding: DP-2 on batch x TP-4 on heads.  Core c = b*4 + r computes
LN1(x_b) over all T tokens, Q/K/V + causal attention for heads 4r..4r+3
only (so K/V projections are not recomputed 4x and score tiles above the
causal diagonal are skipped entirely), then the partial out-projection
for all tokens.  A single bf16 ReduceScatter over each 4-core group sums
the head-group partials and hands core r its 512-token slice, on which it
runs residual + LN2 + the full 4E MLP.

Layouts are feature-major throughout (E on partitions, tokens on the free
axis).  Softmax row sums come free from a ones-column appended to V
(M=65 attnV matmuls).  All weights are host-packed into lhsT layout so
every DMA is contiguous.  Matmuls run in bf16 with fp32 PSUM.
"""

from dataclasses import dataclass

import numpy as np
import ml_dtypes

import concourse.bass as bass  # noqa: F401
import concourse.mybir as mybir
import concourse.tile as tile
from concourse import bacc
from concourse import bass_utils

F32 = mybir.dt.float32
BF16 = mybir.dt.bfloat16
AF = mybir.ActivationFunctionType
OP = mybir.AluOpType
NPBF16 = ml_dtypes.bfloat16

P = 128


@dataclass(frozen=True)
class Cfg:
    B: int = 2
    T: int = 2048
    E: int = 1024
    H: int = 16
    D: int = 64
    NC: int = 8
    eps: float = 1e-5

    @property
    def CPB(self):  # cores per batch (TP group size)
        return self.NC // self.B

    @property
    def Tq(self):  # tokens owned per core (MLP stage)
        return self.T // self.CPB

    @property
    def KE(self):  # E / 128
        return self.E // P

    @property
    def TK(self):  # T / 128 context tiles
        return self.T // P

    @property
    def HPC(self):  # heads per core
        return self.H // self.CPB

    @property
    def JC(self):  # 128-row head-pair blocks per core
        return self.HPC // 2

    @property
    def F(self):
        return 4 * self.E

    @property
    def KF(self):
        return self.F // P

    @property
    def NQC(self):  # query chunks of Tq
        return self.T // self.Tq

    def check(self):
        assert self.D == 64 and self.E == self.H * self.D
        assert self.Tq == 512 and self.HPC == 4 and self.JC == 2
        assert self.T % P == 0 and self.E % P == 0 and self.F % P == 0


CFG = Cfg()


class Pools:
    """Tile pools with explicit open/close (LIFO per side, per space)."""

    def __init__(self, tc, prefix=""):
        self.tc = tc
        self.prefix = prefix
        self.live = {}

    def open(self, key, bufs, space=None, side=None):
        kw = dict(name=self.prefix + key, bufs=bufs)
        if space:
            kw["space"] = space
        if side:
            kw["side"] = side
        cm = self.tc.tile_pool(**kw)
        pool = cm.__enter__()
        self.live[key] = cm
        return pool

    def close(self, *keys):
        for key in keys:
            self.live.pop(key).__exit__(None, None, None)

    def close_all(self):
        for key in reversed(list(self.live)):
            self.close(key)


def _emit(tc, c: Cfg, d):
    nc = tc.nc
    E, T, Tq = c.E, c.T, c.Tq
    KE, TK, KF, JC, NQC, HPC = c.KE, c.TK, c.KF, c.JC, c.NQC, c.HPC
    DV = 65  # V cols per head incl. ones column
    SCL = 1.0 / float(np.sqrt(c.D))

    pp = Pools(tc)

    # ---------------- constants (whole-kernel lifetime) --------------------
    const = pp.open("const", 1)
    ones_bf = const.tile([P, 1], BF16, name="ones_bf")
    nc.vector.memset(ones_bf[:], 1.0)
    ones_f1 = const.tile([1, P], F32, name="ones_f1")
    nc.vector.memset(ones_f1[:], 1.0)
    sel64 = const.tile([4, 4 * 64], BF16, name="sel64")
    nc.sync.dma_start(sel64[:], d["sel64"])
    gbt = {}
    for nm, cols in [
        ("ln1g", KE), ("ln1b", KE), ("ln2g", KE), ("ln2b", KE),
        ("boc", KE), ("mb1", KF), ("mb2", KE),
    ]:
        gbt[nm] = const.tile([P, cols], F32, name=nm + "_t")
        nc.sync.dma_start(gbt[nm][:], d[nm])

    p_band = pp.open("bandp", 1)
    band01 = p_band.tile([P, 4 * 2 * Tq], BF16, name="band01")
    nc.sync.dma_start(band01[:], d["band01"])

    # ---------------- DRAM bounce buffers for the ReduceScatter ------------
    p_dram = pp.open("dram", 1, "DRAM")
    bounce_in = p_dram.tile([NQC * E, Tq], BF16, name="bounce_in")
    bounce_out = p_dram.tile([E, Tq], BF16, name="bounce_out")

    # ---------------- PE warmup (pstate ramp) ------------------------------
    ps_wm = pp.open("warm_ps", 1, "PSUM")
    wmp = ps_wm.tile([1, Tq], F32, name="wmp")
    for _w in range(24):
        nc.tensor.matmul(wmp[:], ones_bf[:], band01[:, 0:Tq], start=True, stop=True)
    pp.close("warm_ps")

    # ======================================================================
    # Phase 0: load x^T (bf16) + LayerNorm1 over all T -> xn (bf16)
    # ======================================================================
    p_kq = pp.open("kqp", 1)
    p_vs = pp.open("vsp", 1)
    qt = [p_kq.tile([P, T], BF16, name=f"qt{j}") for j in range(JC)]
    kt = [p_kq.tile([P, T], BF16, name=f"kt{j}") for j in range(JC)]
    vsb = p_vs.tile([P, TK, HPC, DV], BF16, name="vsb")
    nc.vector.memset(vsb[:, :, :, DV - 1 : DV], 1.0)

    p_xn = pp.open("xnp", 1)
    p_xt = pp.open("xtp", 1)
    xt = [p_xt.tile([P, T], BF16, name=f"xt{e}") for e in range(KE)]
    xn = [p_xn.tile([P, T], BF16, name=f"xn{e}") for e in range(KE)]
    for e in range(KE):
        nc.sync.dma_start(xt[e][:], d["xt"][e * P : (e + 1) * P, :])

    p_tmp = pp.open("ln_tmp", 4)
    p_rows = pp.open("ln_rows", 1)
    ps_st = pp.open("ln_st", 1, "PSUM")
    ps_bc = pp.open("ln_bc", 2, "PSUM")

    for ci in range(NQC):
        cs = slice(ci * Tq, (ci + 1) * Tq)
        s1 = ps_st.tile([1, Tq], F32, name="s1")
        s2 = ps_st.tile([1, Tq], F32, name="s2")
        for e in range(KE):
            x2 = p_tmp.tile([P, Tq], BF16, name="x2bf")
            nc.scalar.square(x2[:], xt[e][:, cs])
            nc.tensor.matmul(s1[:], ones_bf[:], xt[e][:, cs], start=(e == 0), stop=(e == KE - 1))
            nc.tensor.matmul(s2[:], ones_bf[:], x2[:], start=(e == 0), stop=(e == KE - 1))
        mu = p_rows.tile([1, Tq], F32, name="mu")
        nc.vector.tensor_scalar_mul(mu[:], s1[:], 1.0 / E)
        ve = p_rows.tile([1, Tq], F32, name="ve")
        nc.vector.tensor_scalar(ve[:], s2[:], 1.0 / E, c.eps, OP.mult, OP.add)
        mu2 = p_rows.tile([1, Tq], F32, name="mu2")
        nc.vector.tensor_tensor(mu2[:], mu[:], mu[:], OP.mult)
        vee = p_rows.tile([1, Tq], F32, name="vee")
        nc.vector.tensor_tensor(vee[:], ve[:], mu2[:], OP.subtract)
        lv = p_rows.tile([1, Tq], F32, name="lv")
        nc.scalar.activation(lv[:], vee[:], AF.Ln)
        rstd = p_rows.tile([1, Tq], F32, name="rstd")
        nc.scalar.activation(rstd[:], lv[:], AF.Exp, scale=-0.5)

        mub = ps_bc.tile([P, Tq], F32, name="mub")
        nc.tensor.matmul(mub[:], ones_f1[:], mu[:], start=True, stop=True)
        rsb = ps_bc.tile([P, Tq], F32, name="rsb")
        nc.tensor.matmul(rsb[:], ones_f1[:], rstd[:], start=True, stop=True)

        for e in range(KE):
            t1 = p_tmp.tile([P, Tq], F32, name="t1")
            nc.vector.tensor_tensor(t1[:], xt[e][:, cs], mub[:], OP.subtract)
            t2 = p_tmp.tile([P, Tq], F32, name="t2")
            nc.vector.tensor_tensor(t2[:], t1[:], rsb[:], OP.mult)
            nc.vector.tensor_scalar(
                xn[e][:, cs], t2[:],
                gbt["ln1g"][:, e : e + 1], gbt["ln1b"][:, e : e + 1],
                OP.mult, OP.add,
            )
    pp.close("ln_bc", "ln_st", "ln_rows", "ln_tmp", "xtp")

    # ======================================================================
    # Phase 1: Q/K/V projections for this core's 4 heads
    # ======================================================================
    p_w3 = pp.open("w3p", 1)
    wqt = p_w3.tile([P, KE, 2 * P], BF16, name="wqt")
    wkt = p_w3.tile([P, KE, 2 * P], BF16, name="wkt")
    wvt = p_w3.tile([P, KE, 2 * P], BF16, name="wvt")
    nc.sync.dma_start(wqt[:], d["wq"])
    nc.sync.dma_start(wkt[:], d["wk"])
    nc.sync.dma_start(wvt[:], d["wv"])

    ps_qkv = pp.open("qkv_ps", 2, "PSUM")

    for j in range(JC):
        for w_t, dst in ((wkt, kt), (wqt, qt)):
            for ci in range(NQC):
                cs = slice(ci * Tq, (ci + 1) * Tq)
                ps = ps_qkv.tile([P, Tq], F32, name="kq_ps")
                for e in range(KE):
                    nc.tensor.matmul(
                        ps[:], w_t[:, e, j * P : (j + 1) * P], xn[e][:, cs],
                        start=(e == 0), stop=(e == KE - 1),
                    )
                nc.vector.tensor_copy(dst[j][:, cs], ps[:])

    for t in range(TK):
        ps = ps_qkv.tile([P, 2 * P], F32, name="v_ps")
        for e in range(KE):
            nc.tensor.matmul(
                ps[:], xn[e][:, t * P : (t + 1) * P], wvt[:, e, :],
                start=(e == 0), stop=(e == KE - 1),
            )
        nc.vector.tensor_copy(
            vsb[:, t, :, 0:64],
            ps[:].rearrange("p (h v) -> p h v", h=HPC),
        )
    pp.close("qkv_ps", "w3p", "xnp")

    # prefetch the out-proj + MLP weights + residual slice while attention runs
    p_wo = pp.open("wop", 1, side="right")
    wot = p_wo.tile([P, JC, E], BF16, name="wot")
    nc.sync.dma_start(wot[:], d["wo"])
    p_w12 = pp.open("w12p", 1, side="right")
    w1t = p_w12.tile([P, KE, c.F], BF16, name="w1t")
    w2t = p_w12.tile([P, KF, E], BF16, name="w2t")
    nc.sync.dma_start(w1t[:], d["w1"])
    nc.sync.dma_start(w2t[:], d["w2"])
    # ======================================================================
    # Phase 2: causal attention for 4 heads (2 pairs), all query chunks
    # ======================================================================
    p_ao = pp.open("aop", 1)
    p_pr = pp.open("probs", 2)
    p_rst = pp.open("rsst", 1)
    p_st2 = pp.open("rstage", 2)
    ps_av = pp.open("av_ps", 1, "PSUM")
    ps_ss = pp.open("ss_ps", 2, "PSUM")

    for qc in range(NQC):
        qs = slice(qc * Tq, (qc + 1) * Tq)
        ntile = 4 * qc + 4
        avp = [ps_av.tile([DV, Tq], F32, name=f"avp{h}") for h in range(HPC)]
        for t in range(ntile):
            for p in range(JC):
                ss = ps_ss.tile([P, 2 * Tq], F32, name="ss")
                for s in (0, 1):
                    nc.tensor.matmul(
                        ss[:, s * Tq : (s + 1) * Tq],
                        kt[p][s * 64 : (s + 1) * 64, t * P : (t + 1) * P],
                        qt[p][s * 64 : (s + 1) * 64, qs],
                        start=True, stop=True,
                        tile_position=(s * 64, 0),
                    )
                pr = p_pr.tile([P, 2 * Tq], BF16, name="pr")
                nc.scalar.activation(pr[:], ss[:], AF.Exp, scale=SCL)
                jb = t - 4 * qc
                if jb >= 0:
                    nc.vector.tensor_tensor(
                        pr[:], pr[:],
                        band01[:, jb * 2 * Tq : (jb + 1) * 2 * Tq], OP.mult,
                    )
                for s in (0, 1):
                    h = 2 * p + s
                    nc.tensor.matmul(
                        avp[h][:],
                        vsb[:, t, h, :],
                        pr[:, s * Tq : (s + 1) * Tq],
                        start=(t == 0), stop=(t == ntile - 1),
                        skip_group_check=True,
                    )

        # softmax denominators: rowsums sit on partition 64 of each avp
        st = p_rst.tile([P, HPC * Tq], F32, name="rs_st")
        for h in range(HPC):
            nc.vector.tensor_copy(
                st[64:65, h * Tq : (h + 1) * Tq], avp[h][64:65, :]
            )
        rs4 = p_st2.tile([4, Tq], F32, name="rs4", bufs=1)
        nc.sync.dma_start(rs4[:], st[64:65, :])
        rec4 = p_st2.tile([4, Tq], F32, name="rec4", bufs=1)
        nc.vector.reciprocal(rec4[:], rs4[:])
        irs = p_st2.tile([4, Tq], BF16, name="irs", bufs=1)
        nc.vector.tensor_copy(irs[:], rec4[:])

        aop = [p_ao.tile([P, Tq], BF16, name=f"aop{p}") for p in range(JC)]
        for p in range(JC):
            nb = ps_ss.tile([P, 2 * Tq], F32, name="ss")
            for s in (0, 1):
                nc.tensor.matmul(
                    nb[0:64, s * Tq : (s + 1) * Tq],
                    sel64[:, (2 * p + s) * 64 : (2 * p + s + 1) * 64],
                    irs[:],
                    start=True, stop=True,
                    skip_group_check=True,
                )
            nbs = p_st2.tile([64, 2 * Tq], BF16, name="nbs", bufs=1)
            nc.scalar.copy(nbs[:], nb[0:64, :])
            nc.vector.tensor_tensor(
                aop[p][0:64, :], avp[2 * p][0:64, :], nbs[:, 0:Tq], OP.mult
            )
            ost = p_st2.tile([64, Tq], BF16, name="ost")
            nc.vector.tensor_tensor(
                ost[:], avp[2 * p + 1][0:64, :], nbs[:, Tq : 2 * Tq], OP.mult
            )
            nc.sync.dma_start(aop[p][64:128, :], ost[:])

        # partial out-projection for this chunk -> bounce_in block qc
        for e in range(KE):
            po = ps_ss.tile([P, 2 * Tq], F32, name="ss")
            for p in range(JC):
                nc.tensor.matmul(
                    po[:, 0:Tq], wot[:, p, e * P : (e + 1) * P], aop[p][:],
                    start=(p == 0), stop=(p == JC - 1),
                    skip_group_check=True,
                )
            ob = p_st2.tile([P, Tq], BF16, name="ob")
            nc.vector.tensor_copy(ob[:], po[:, 0:Tq])
            nc.sync.dma_start(
                bounce_in[(qc * KE + e) * P : (qc * KE + e + 1) * P, :], ob[:]
            )

    pp.close("ss_ps", "av_ps", "rstage", "rsst", "probs", "aop")
    pp.close("vsp", "kqp", "bandp")

    # ======================================================================
    # Phase 3: ReduceScatter partials; residual + bo -> xres; LN2 -> xn2
    # ======================================================================
    p_xo = pp.open("xop", 1, side="right")
    xown = [p_xo.tile([P, Tq], F32, name=f"xown{e}") for e in range(KE)]
    for e in range(KE):
        nc.sync.dma_start(xown[e][:], d["x_own"][e * P : (e + 1) * P, :])

    nc.gpsimd.collective_compute(
        "ReduceScatter",
        OP.add,
        replica_groups=[[0, 1, 2, 3], [4, 5, 6, 7]],
        ins=[bounce_in.opt()],
        outs=[bounce_out.opt()],
    )

    p_xr = pp.open("xrp", 1)
    p_x2 = pp.open("xn2p", 1)
    p_att = pp.open("attp", 1)
    xres = [p_xr.tile([P, Tq], F32, name=f"xres{e}") for e in range(KE)]
    xn2 = [p_x2.tile([P, Tq], BF16, name=f"xn2{e}") for e in range(KE)]
    att = [p_att.tile([P, Tq], BF16, name=f"att{e}") for e in range(KE)]
    for e in range(KE):
        nc.sync.dma_start(att[e][:], bounce_out[e * P : (e + 1) * P, :])
        nc.vector.scalar_tensor_tensor(
            xres[e][:], att[e][:], gbt["boc"][:, e : e + 1], xown[e][:],
            OP.add, OP.add,
        )
    pp.close("attp")
    pp.close("xop")

    p_tmp = pp.open("ln2_tmp", 3)
    p_rows = pp.open("ln2_rows", 1)
    ps_st = pp.open("ln2_st", 1, "PSUM")
    ps_bc = pp.open("ln2_bc", 1, "PSUM")
    s1 = ps_st.tile([1, Tq], F32, name="s1b")
    s2 = ps_st.tile([1, Tq], F32, name="s2b")
    for e in range(KE):
        xbf = p_tmp.tile([P, Tq], BF16, name="xbf2")
        nc.vector.tensor_copy(xbf[:], xres[e][:])
        x2 = p_tmp.tile([P, Tq], BF16, name="x2bf2")
        nc.scalar.square(x2[:], xres[e][:])
        nc.tensor.matmul(s1[:], ones_bf[:], xbf[:], start=(e == 0), stop=(e == KE - 1))
        nc.tensor.matmul(s2[:], ones_bf[:], x2[:], start=(e == 0), stop=(e == KE - 1))
    mu = p_rows.tile([1, Tq], F32, name="mu_2")
    nc.vector.tensor_scalar_mul(mu[:], s1[:], 1.0 / E)
    ve = p_rows.tile([1, Tq], F32, name="ve_2")
    nc.vector.tensor_scalar(ve[:], s2[:], 1.0 / E, c.eps, OP.mult, OP.add)
    mu2 = p_rows.tile([1, Tq], F32, name="mu2_2")
    nc.vector.tensor_tensor(mu2[:], mu[:], mu[:], OP.mult)
    vee = p_rows.tile([1, Tq], F32, name="vee_2")
    nc.vector.tensor_tensor(vee[:], ve[:], mu2[:], OP.subtract)
    lv = p_rows.tile([1, Tq], F32, name="lv_2")
    nc.scalar.activation(lv[:], vee[:], AF.Ln)
    rstd = p_rows.tile([1, Tq], F32, name="rstd_2")
    nc.scalar.activation(rstd[:], lv[:], AF.Exp, scale=-0.5)
    mub = ps_bc.tile([P, Tq], F32, name="mub2")
    nc.tensor.matmul(mub[:], ones_f1[:], mu[:], start=True, stop=True)
    rsb = ps_bc.tile([P, Tq], F32, name="rsb2")
    nc.tensor.matmul(rsb[:], ones_f1[:], rstd[:], start=True, stop=True)
    for e in range(KE):
        t1 = p_tmp.tile([P, Tq], F32, name="t1b")
        nc.vector.tensor_tensor(t1[:], xres[e][:], mub[:], OP.subtract)
        t2 = p_tmp.tile([P, Tq], F32, name="t2b")
        nc.vector.tensor_tensor(t2[:], t1[:], rsb[:], OP.mult)
        nc.vector.tensor_scalar(
            xn2[e][:], t2[:],
            gbt["ln2g"][:, e : e + 1], gbt["ln2b"][:, e : e + 1],
            OP.mult, OP.add,
        )
    pp.close("ln2_bc", "ln2_st", "ln2_rows", "ln2_tmp")

    # ======================================================================
    # Phase 4: MLP (layer 1 streamed with first 6 e-tiles of layer 2)
    # ======================================================================
    EH = min(KE, 6)
    p_h1 = pp.open("h1p", 1, side="right")
    p_out = pp.open("outp", 2)
    ps_h1 = pp.open("h1_ps", 2, "PSUM")
    ps_h2a = pp.open("h2a_ps", 1, "PSUM")

    h1 = [p_h1.tile([P, Tq], BF16, name=f"h1{f}") for f in range(KF)]
    h2a = [ps_h2a.tile([P, Tq], F32, name=f"h2a{e}") for e in range(EH)]
    for f in range(KF):
        ps = ps_h1.tile([P, Tq], F32, name="h1ps")
        for e in range(KE):
            nc.tensor.matmul(
                ps[:], w1t[:, e, f * P : (f + 1) * P], xn2[e][:],
                start=(e == 0), stop=(e == KE - 1),
            )
        nc.scalar.activation(
            h1[f][:], ps[:], AF.Relu, bias=gbt["mb1"][:, f : f + 1], scale=1.0
        )
        for e in range(EH):
            nc.tensor.matmul(
                h2a[e][:], w2t[:, f, e * P : (e + 1) * P], h1[f][:],
                start=(f == 0), stop=(f == KF - 1),
            )
    for e in range(EH):
        of = p_out.tile([P, Tq], F32, name="outf")
        nc.vector.scalar_tensor_tensor(
            of[:], h2a[e][:], gbt["mb2"][:, e : e + 1], xres[e][:], OP.add, OP.add
        )
        nc.sync.dma_start(d["out_t"][e * P : (e + 1) * P, :], of[:])
    pp.close("h2a_ps", "h1_ps")

    if EH < KE:
        ps_h2b = pp.open("h2b_ps", 1, "PSUM")
        h2b = [ps_h2b.tile([P, Tq], F32, name=f"h2b{e}") for e in range(KE - EH)]
        for f in range(KF):
            for i, e in enumerate(range(EH, KE)):
                nc.tensor.matmul(
                    h2b[i][:], w2t[:, f, e * P : (e + 1) * P], h1[f][:],
                    start=(f == 0), stop=(f == KF - 1),
                )
        for i, e in enumerate(range(EH, KE)):
            of = p_out.tile([P, Tq], F32, name="outf")
            nc.vector.scalar_tensor_tensor(
                of[:], h2b[i][:], gbt["mb2"][:, e : e + 1], xres[e][:], OP.add, OP.add
            )
            nc.sync.dma_start(d["out_t"][e * P : (e + 1) * P, :], of[:])

    pp.close_all()


def build_program(c: Cfg = CFG):
    c.check()
    nc = bacc.Bacc(
        "TRN2",
        target_bir_lowering=False,
        debug=False,
        enable_asserts=False,
        num_devices=c.NC,
    )
    d = {}
    d["xt"] = nc.dram_tensor("xt", [c.E, c.T], BF16, kind="ExternalInput").ap()
    d["x_own"] = nc.dram_tensor("x_own", [c.E, c.Tq], F32, kind="ExternalInput").ap()
    d["wq"] = nc.dram_tensor("wq", [P, c.KE, 2 * P], BF16, kind="ExternalInput").ap()
    d["wk"] = nc.dram_tensor("wk", [P, c.KE, 2 * P], BF16, kind="ExternalInput").ap()
    d["wv"] = nc.dram_tensor("wv", [P, c.KE, 2 * P], BF16, kind="ExternalInput").ap()
    d["wo"] = nc.dram_tensor("wo", [P, c.JC, c.E], BF16, kind="ExternalInput").ap()
    d["w1"] = nc.dram_tensor("w1", [P, c.KE, c.F], BF16, kind="ExternalInput").ap()
    d["w2"] = nc.dram_tensor("w2", [P, c.KF, c.E], BF16, kind="ExternalInput").ap()
    for nm, cols in [
        ("ln1g", c.KE), ("ln1b", c.KE), ("ln2g", c.KE), ("ln2b", c.KE),
        ("boc", c.KE), ("mb1", c.KF), ("mb2", c.KE),
    ]:
        d[nm] = nc.dram_tensor(nm, [P, cols], F32, kind="ExternalInput").ap()
    d["band01"] = nc.dram_tensor(
        "band01", [P, 4 * 2 * c.Tq], BF16, kind="ExternalInput"
    ).ap()
    d["sel64"] = nc.dram_tensor("sel64", [4, 4 * 64], BF16, kind="ExternalInput").ap()
    d["out_t"] = nc.dram_tensor("out_t", [c.E, c.Tq], F32, kind="ExternalOutput").ap()

    with tile.TileContext(nc) as tc:
        _emit(tc, c, d)
    nc.compile()
    return nc


# --------------------------------------------------------------------------
# host side
# --------------------------------------------------------------------------
def _pack_lhsT(w, cols_per_block):
    """[R, C] -> [128, R//128, C] lhsT layout (contiguous DMA)."""
    R, C = w.shape
    assert R % P == 0 and C == cols_per_block
    return np.ascontiguousarray(
        w.reshape(R // P, P, C).transpose(1, 0, 2)
    )


def shard_inputs(inputs, c: Cfg = CFG):
    x = np.ascontiguousarray(np.asarray(inputs["x"], np.float32))
    bf = lambda a: np.ascontiguousarray(np.asarray(a, np.float32)).astype(NPBF16)

    chunks = lambda v, k: np.ascontiguousarray(
        np.asarray(v, np.float32).reshape(k, P).T
    )
    com = {
        "w1": _pack_lhsT(bf(inputs["W1"]), c.F),
        "w2": _pack_lhsT(bf(inputs["W2"]), c.E),
        "ln1g": chunks(inputs["ln1_g"], c.KE),
        "ln1b": chunks(inputs["ln1_b"], c.KE),
        "ln2g": chunks(inputs["ln2_g"], c.KE),
        "ln2b": chunks(inputs["ln2_b"], c.KE),
        "boc": chunks(inputs["bo"], c.KE),
        "mb1": chunks(inputs["b1"], c.KF),
        "mb2": chunks(inputs["b2"], c.KE),
    }

    p_idx = np.arange(P)[:, None]
    tq_idx = np.arange(c.Tq)[None, :]
    band = np.zeros((P, 4 * 2 * c.Tq), np.float32)
    for jb in range(4):
        m = (tq_idx >= (jb * P + p_idx)).astype(np.float32)
        for s in range(2):
            band[:, jb * 2 * c.Tq + s * c.Tq : jb * 2 * c.Tq + (s + 1) * c.Tq] = m
    com["band01"] = band.astype(NPBF16)

    sel = np.zeros((4, 4 * 64), np.float32)
    for h in range(4):
        sel[h, h * 64 : (h + 1) * 64] = 1.0
    com["sel64"] = sel.astype(NPBF16)

    Wq, Wk, Wv = (bf(inputs[k]) for k in ("Wq", "Wk", "Wv"))
    Wo = bf(inputs["Wo"])
    maps = []
    for core in range(c.NC):
        b, r = core // c.CPB, core % c.CPB
        hs = slice(r * 2 * P, (r + 1) * 2 * P)  # this core's 256 head-features
        m = dict(com)
        m["xt"] = np.ascontiguousarray(x[b].T).astype(NPBF16)
        m["x_own"] = np.ascontiguousarray(x[b, r * c.Tq : (r + 1) * c.Tq, :].T)
        m["wq"] = _pack_lhsT(Wq[:, hs], 2 * P)
        m["wk"] = _pack_lhsT(Wk[:, hs], 2 * P)
        m["wv"] = _pack_lhsT(Wv[:, hs], 2 * P)
        m["wo"] = _pack_lhsT(np.ascontiguousarray(Wo[hs, :]), c.E)
        maps.append(m)
    return maps


def assemble(results, c: Cfg = CFG):
    out = np.empty((c.B, c.T, c.E), np.float32)
    for core in range(c.NC):
        b, r = core // c.CPB, core % c.CPB
        out[b, r * c.Tq : (r + 1) * c.Tq, :] = results[core]["out_t"].T
    return out


_NC_CACHE = {}


def _get_nc(c: Cfg = CFG):
    if c not in _NC_CACHE:
        _NC_CACHE[c] = build_program(c)
    return _NC_CACHE[c]


LAST_RESULT = None


def kernel(**inputs):
    global LAST_RESULT
    c = CFG
    nc = _get_nc(c)
    maps = shard_inputs(inputs, c)
    res = bass_utils.run_bass_kernel_spmd(nc, maps, core_ids=list(range(c.NC)))
    LAST_RESULT = res
    return assemble(res.results, c)


# revision 19
# speedup vs baseline: 1.2561x; 1.0963x over previous
"""Trainium2 Bass kernel for a pre-norm transformer block (dense_transformer).

Computation (per reference):
    x = x + Attn(LN1(x));  x = x + MLP(LN2(x))
with causal multi-head attention (H=16 heads, D=64) and a 4E ReLU MLP.

Sharding: DP-2 on batch x TP-4 on heads.  Core c = b*4 + r computes
LN1(x_b) over all T tokens, Q/K/V + causal attention for heads 4r..4r+3
only (so K/V projections are not recomputed 4x and score tiles above the
causal diagonal are skipped entirely), then the partial out-projection
for all tokens.  A single bf16 ReduceScatter over each 4-core group sums
the head-group partials and hands core r its 512-token slice, on which it
runs residual + LN2 + the full 4E MLP.

Layouts are feature-major throughout (E on partitions, tokens on the free
axis).  Softmax row sums come free from a ones-column appended to V
(M=65 attnV matmuls).  All weights are host-packed into lhsT layout so
every DMA is contiguous.  Matmuls run in bf16 with fp32 PSUM.
"""

from dataclasses import dataclass

import numpy as np
import ml_dtypes

import concourse.bass as bass  # noqa: F401
import concourse.mybir as mybir
import concourse.tile as tile
from concourse import bacc
from concourse import bass_utils

F32 = mybir.dt.float32
BF16 = mybir.dt.bfloat16
AF = mybir.ActivationFunctionType
OP = mybir.AluOpType
NPBF16 = ml_dtypes.bfloat16

P = 128


@dataclass(frozen=True)
class Cfg:
    B: int = 2
    T: int = 2048
    E: int = 1024
    H: int = 16
    D: int = 64
    NC: int = 8
    eps: float = 1e-5

    @property
    def CPB(self):  # cores per batch (TP group size)
        return self.NC // self.B

    @property
    def Tq(self):  # tokens owned per core (MLP stage)
        return self.T // self.CPB

    @property
    def KE(self):  # E / 128
        return self.E // P

    @property
    def TK(self):  # T / 128 context tiles
        return self.T // P

    @property
    def HPC(self):  # heads per core
        return self.H // self.CPB

    @property
    def JC(self):  # 128-row head-pair blocks per core
        return self.HPC // 2

    @property
    def F(self):
        return 4 * self.E

    @property
    def KF(self):
        return self.F // P

    @property
    def NQC(self):  # query chunks of Tq
        return self.T // self.Tq

    def check(self):
        assert self.D == 64 and self.E == self.H * self.D
        assert self.Tq == 512 and self.HPC == 4 and self.JC == 2
        assert self.T % P == 0 and self.E % P == 0 and self.F % P == 0


CFG = Cfg()


class Pools:
    """Tile pools with explicit open/close (LIFO per side, per space)."""

    def __init__(self, tc, prefix=""):
        self.tc = tc
        self.prefix = prefix
        self.live = {}

    def open(self, key, bufs, space=None, side=None):
        kw = dict(name=self.prefix + key, bufs=bufs)
        if space:
            kw["space"] = space
        if side:
            kw["side"] = side
        cm = self.tc.tile_pool(**kw)
        pool = cm.__enter__()
        self.live[key] = cm
        return pool

    def close(self, *keys):
        for key in keys:
            self.live.pop(key).__exit__(None, None, None)

    def close_all(self):
        for key in reversed(list(self.live)):
            self.close(key)


def _emit(tc, c: Cfg, d):
    nc = tc.nc
    E, T, Tq = c.E, c.T, c.Tq
    KE, TK, KF, JC, NQC, HPC = c.KE, c.TK, c.KF, c.JC, c.NQC, c.HPC
    DV = 65  # V cols per head incl. ones column
    SCL = 1.0 / float(np.sqrt(c.D))

    pp = Pools(tc)

    # ---------------- constants (whole-kernel lifetime) --------------------
    const = pp.open("const", 1)
    ones_bf = const.tile([P, 1], BF16, name="ones_bf")
    nc.vector.memset(ones_bf[:], 1.0)
    ones_f1 = const.tile([1, P], F32, name="ones_f1")
    nc.vector.memset(ones_f1[:], 1.0)
    sel64 = const.tile([4, 4 * 64], BF16, name="sel64")
    nc.sync.dma_start(sel64[:], d["sel64"])
    gbt = {}
    for nm, cols in [
        ("ln1g", KE), ("ln1b", KE), ("ln2g", KE), ("ln2b", KE),
        ("boc", KE), ("mb1", KF), ("mb2", KE),
    ]:
        gbt[nm] = const.tile([P, cols], F32, name=nm + "_t")
        nc.sync.dma_start(gbt[nm][:], d[nm])

    ident = const.tile([P, P], BF16, name="ident")
    nc.sync.dma_start(ident[:], d["ident"])
    wrm = const.tile([P, Tq], BF16, name="wrm")
    nc.vector.memset(wrm[:], 0.0)

    p_band = pp.open("bandp", 1)
    mask01 = p_band.tile([P, 4 * Tq], BF16, name="mask01")
    nc.sync.dma_start(mask01[:], d["mask01"])

    # ---------------- DRAM bounce buffers for the ReduceScatter ------------
    p_dram = pp.open("dram", 1, "DRAM")
    bounce_in = p_dram.tile([NQC * E, Tq], BF16, name="bounce_in")
    bounce_out = p_dram.tile([E, Tq], BF16, name="bounce_out")

    # ---------------- PE warmup (pstate ramp) ------------------------------
    ps_wm = pp.open("warm_ps", 1, "PSUM")
    wmp = ps_wm.tile([1, Tq], F32, name="wmp")
    for _w in range(24):
        nc.tensor.matmul(wmp[:], ones_bf[:], wrm[:], start=True, stop=True)
    pp.close("warm_ps")

    # ======================================================================
    # Phase 0: load x^T (bf16) + LayerNorm1 over all T -> xn (bf16)
    # ======================================================================
    p_kq = pp.open("kqp", 1)
    p_vs = pp.open("vsp", 1)
    qt = [p_kq.tile([P, T], BF16, name=f"qt{j}") for j in range(JC)]
    kt = [p_kq.tile([P, T], BF16, name=f"kt{j}") for j in range(JC)]
    vsb = p_vs.tile([P, TK, HPC, DV], BF16, name="vsb")
    nc.vector.memset(vsb[:, :, :, DV - 1 : DV], 1.0)

    p_xn = pp.open("xnp", 1)
    p_xt = pp.open("xtp", 1)
    xt = [p_xt.tile([P, T], BF16, name=f"xt{e}") for e in range(KE)]
    xn = [p_xn.tile([P, T], BF16, name=f"xn{e}") for e in range(KE)]
    for e in range(KE):
        nc.sync.dma_start(xt[e][:], d["xt"][e * P : (e + 1) * P, :])

    p_tmp = pp.open("ln_tmp", 4)
    p_rows = pp.open("ln_rows", 1)
    p_bcs = pp.open("ln_bcs", 2)
    ps_st = pp.open("ln_st", 1, "PSUM")

    for ci in range(NQC):
        cs = slice(ci * Tq, (ci + 1) * Tq)
        s1 = ps_st.tile([1, Tq], F32, name="s1")
        s2 = ps_st.tile([1, Tq], F32, name="s2")
        for e in range(KE):
            x2 = p_tmp.tile([P, Tq], BF16, name="x2bf")
            nc.scalar.square(x2[:], xt[e][:, cs])
            nc.tensor.matmul(s1[:], ones_bf[:], xt[e][:, cs], start=(e == 0), stop=(e == KE - 1))
            nc.tensor.matmul(s2[:], ones_bf[:], x2[:], start=(e == 0), stop=(e == KE - 1))
        mu = p_rows.tile([1, Tq], F32, name="mu")
        nc.vector.tensor_scalar_mul(mu[:], s1[:], 1.0 / E)
        ve = p_rows.tile([1, Tq], F32, name="ve")
        nc.vector.tensor_scalar(ve[:], s2[:], 1.0 / E, c.eps, OP.mult, OP.add)
        mu2 = p_rows.tile([1, Tq], F32, name="mu2")
        nc.vector.tensor_tensor(mu2[:], mu[:], mu[:], OP.mult)
        vee = p_rows.tile([1, Tq], F32, name="vee")
        nc.vector.tensor_tensor(vee[:], ve[:], mu2[:], OP.subtract)
        lv = p_rows.tile([1, Tq], F32, name="lv")
        nc.scalar.activation(lv[:], vee[:], AF.Ln)
        rstd = p_rows.tile([1, Tq], F32, name="rstd")
        nc.scalar.activation(rstd[:], lv[:], AF.Exp, scale=-0.5)

        mub = p_bcs.tile([P, Tq], F32, name="mub")
        nc.gpsimd.partition_broadcast(mub[:], mu[:])
        rsb = p_bcs.tile([P, Tq], F32, name="rsb")
        nc.gpsimd.partition_broadcast(rsb[:], rstd[:])

        for e in range(KE):
            # alternate DVE / gpsimd so the normalize stream isn't one-engine
            eng = nc.vector if e % 2 == 0 else nc.gpsimd
            t1 = p_tmp.tile([P, Tq], F32, name=f"t1_{e % 2}")
            eng.tensor_tensor(t1[:], xt[e][:, cs], mub[:], OP.subtract)
            t2 = p_tmp.tile([P, Tq], F32, name=f"t2_{e % 2}")
            eng.tensor_tensor(t2[:], t1[:], rsb[:], OP.mult)
            eng.tensor_scalar(
                xn[e][:, cs], t2[:],
                gbt["ln1g"][:, e : e + 1], gbt["ln1b"][:, e : e + 1],
                OP.mult, OP.add,
            )
    pp.close("ln_st", "ln_bcs", "ln_rows", "ln_tmp", "xtp")

    # ======================================================================
    # Phase 1: Q/K/V projections for this core's 4 heads
    # ======================================================================
    p_w3 = pp.open("w3p", 1)
    wqt = p_w3.tile([P, KE, 2 * P], BF16, name="wqt")
    wkt = p_w3.tile([P, KE, 2 * P], BF16, name="wkt")
    wvt = p_w3.tile([P, KE, 2 * P], BF16, name="wvt")
    nc.sync.dma_start(wqt[:], d["wq"])
    nc.sync.dma_start(wkt[:], d["wk"])
    nc.sync.dma_start(wvt[:], d["wv"])

    ps_qkv = pp.open("qkv_ps", 2, "PSUM")

    for j in range(JC):
        for w_t, dst in ((wkt, kt), (wqt, qt)):
            for ci in range(NQC):
                cs = slice(ci * Tq, (ci + 1) * Tq)
                ps = ps_qkv.tile([P, Tq], F32, name="kq_ps")
                for e in range(KE):
                    nc.tensor.matmul(
                        ps[:], w_t[:, e, j * P : (j + 1) * P], xn[e][:, cs],
                        start=(e == 0), stop=(e == KE - 1),
                    )
                nc.scalar.copy(dst[j][:, cs], ps[:])

    for t in range(TK):
        ps = ps_qkv.tile([P, 2 * P], F32, name="v_ps")
        for e in range(KE):
            nc.tensor.matmul(
                ps[:], xn[e][:, t * P : (t + 1) * P], wvt[:, e, :],
                start=(e == 0), stop=(e == KE - 1),
            )
        nc.scalar.copy(
            vsb[:, t, :, 0:64],
            ps[:].rearrange("p (h v) -> p h v", h=HPC),
        )
    pp.close("qkv_ps", "w3p", "xnp")

    # prefetch the out-proj + MLP weights + residual slice while attention runs
    p_wo = pp.open("wop", 1, side="right")
    wot = p_wo.tile([P, JC, E], BF16, name="wot")
    nc.sync.dma_start(wot[:], d["wo"])
    p_w12 = pp.open("w12p", 1, side="right")
    w1t = p_w12.tile([P, KE, c.F], BF16, name="w1t")
    w2t = p_w12.tile([P, KF, E], BF16, name="w2t")
    nc.sync.dma_start(w1t[:], d["w1"])
    nc.sync.dma_start(w2t[:], d["w2"])
    # ======================================================================
    # Phase 2: causal attention for 4 heads (2 pairs), all query chunks
    # ======================================================================
    p_ao = pp.open("aop", 1)
    p_pr = pp.open("probs", 4)
    p_rst = pp.open("rsst", 1)
    p_st2 = pp.open("rstage", 2)
    ps_av = pp.open("av_ps", 1, "PSUM")
    ps_ss = pp.open("ss_ps", 2, "PSUM")

    for qc in range(NQC):
        qs = slice(qc * Tq, (qc + 1) * Tq)
        ntile = 4 * qc + 4
        avp = [ps_av.tile([DV, Tq], F32, name=f"avp{h}") for h in range(HPC)]
        units = [(t, p) for t in range(ntile) for p in range(JC)]

        def emit_ss(i):
            t, p = units[i]
            jb = t - 4 * qc  # >= 0 on the causal diagonal band
            ssu = ps_ss.tile([P, 2 * Tq], F32, name="ss")
            for s in (0, 1):
                nc.tensor.matmul(
                    ssu[:, s * Tq : (s + 1) * Tq],
                    kt[p][s * 64 : (s + 1) * 64, t * P : (t + 1) * P],
                    qt[p][s * 64 : (s + 1) * 64, qs],
                    start=True, stop=(jb < 0),
                    tile_position=(s * 64, 0),
                    skip_group_check=True,
                )
            if jb >= 0:
                # add -1e9 above the diagonal straight into the score PSUM
                for s in (0, 1):
                    nc.tensor.matmul(
                        ssu[:, s * Tq : (s + 1) * Tq], ident[:],
                        mask01[:, jb * Tq : (jb + 1) * Tq],
                        start=False, stop=True,
                        skip_group_check=True,
                    )
            return ssu

        ss_cur = emit_ss(0)
        for i, (t, p) in enumerate(units):
            prs = []
            for s in (0, 1):
                pru = p_pr.tile([P, Tq], BF16, name="pr")
                nc.scalar.activation(
                    pru[:], ss_cur[:, s * Tq : (s + 1) * Tq], AF.Exp, scale=SCL
                )
                prs.append(pru)
            ss_nxt = emit_ss(i + 1) if i + 1 < len(units) else None
            for s in (0, 1):
                h = 2 * p + s
                nc.tensor.matmul(
                    avp[h][:],
                    vsb[:, t, h, :],
                    prs[s][:],
                    start=(t == 0), stop=(t == ntile - 1),
                    skip_group_check=True,
                )
            ss_cur = ss_nxt

        # softmax denominators: rowsums sit on partition 64 of each avp
        st = p_rst.tile([P, HPC * Tq], F32, name="rs_st")
        for h in range(HPC):
            nc.vector.tensor_copy(
                st[64:65, h * Tq : (h + 1) * Tq], avp[h][64:65, :]
            )
        rs4 = p_st2.tile([4, Tq], F32, name="rs4", bufs=1)
        nc.sync.dma_start(rs4[:], st[64:65, :])
        rec4 = p_st2.tile([4, Tq], F32, name="rec4", bufs=1)
        nc.vector.reciprocal(rec4[:], rs4[:])
        irs = p_st2.tile([4, Tq], BF16, name="irs", bufs=1)
        nc.vector.tensor_copy(irs[:], rec4[:])

        aop = [p_ao.tile([P, Tq], BF16, name=f"aop{p}") for p in range(JC)]
        for p in range(JC):
            nb = ps_ss.tile([P, 2 * Tq], F32, name="ss")
            for s in (0, 1):
                nc.tensor.matmul(
                    nb[0:64, s * Tq : (s + 1) * Tq],
                    sel64[:, (2 * p + s) * 64 : (2 * p + s + 1) * 64],
                    irs[:],
                    start=True, stop=True,
                    skip_group_check=True,
                )
            nbs = p_st2.tile([64, 2 * Tq], BF16, name="nbs", bufs=1)
            nc.scalar.copy(nbs[:], nb[0:64, :])
            nc.vector.tensor_tensor(
                aop[p][0:64, :], avp[2 * p][0:64, :], nbs[:, 0:Tq], OP.mult
            )
            ost = p_st2.tile([64, Tq], BF16, name="ost")
            nc.vector.tensor_tensor(
                ost[:], avp[2 * p + 1][0:64, :], nbs[:, Tq : 2 * Tq], OP.mult
            )
            nc.sync.dma_start(aop[p][64:128, :], ost[:])

        # partial out-projection for this chunk -> bounce_in block qc
        for e in range(KE):
            po = ps_ss.tile([P, 2 * Tq], F32, name="ss")
            for p in range(JC):
                nc.tensor.matmul(
                    po[:, 0:Tq], wot[:, p, e * P : (e + 1) * P], aop[p][:],
                    start=(p == 0), stop=(p == JC - 1),
                    skip_group_check=True,
                )
            ob = p_st2.tile([P, Tq], BF16, name="ob")
            nc.vector.tensor_copy(ob[:], po[:, 0:Tq])
            nc.sync.dma_start(
                bounce_in[(qc * KE + e) * P : (qc * KE + e + 1) * P, :], ob[:]
            )

    pp.close("ss_ps", "av_ps", "rstage", "rsst", "probs", "aop")
    pp.close("vsp", "kqp", "bandp")

    # ======================================================================
    # Phase 3: ReduceScatter partials; residual + bo -> xres; LN2 -> xn2
    # ======================================================================
    p_xo = pp.open("xop", 1, side="right")
    xown = [p_xo.tile([P, Tq], F32, name=f"xown{e}") for e in range(KE)]
    for e in range(KE):
        nc.sync.dma_start(xown[e][:], d["x_own"][e * P : (e + 1) * P, :])

    nc.gpsimd.collective_compute(
        "ReduceScatter",
        OP.add,
        replica_groups=[[0, 1, 2, 3], [4, 5, 6, 7]],
        ins=[bounce_in.opt()],
        outs=[bounce_out.opt()],
    )

    p_xr = pp.open("xrp", 1)
    p_x2 = pp.open("xn2p", 1)
    p_att = pp.open("attp", 1)
    xres = [p_xr.tile([P, Tq], F32, name=f"xres{e}") for e in range(KE)]
    xn2 = [p_x2.tile([P, Tq], BF16, name=f"xn2{e}") for e in range(KE)]
    att = [p_att.tile([P, Tq], BF16, name=f"att{e}") for e in range(KE)]
    for e in range(KE):
        nc.sync.dma_start(att[e][:], bounce_out[e * P : (e + 1) * P, :])
        nc.vector.scalar_tensor_tensor(
            xres[e][:], att[e][:], gbt["boc"][:, e : e + 1], xown[e][:],
            OP.add, OP.add,
        )
    pp.close("attp")
    pp.close("xop")

    p_tmp = pp.open("ln2_tmp", 3)
    p_rows = pp.open("ln2_rows", 1)
    p_bc2 = pp.open("ln2_bcs", 1)
    ps_st = pp.open("ln2_st", 1, "PSUM")
    s1 = ps_st.tile([1, Tq], F32, name="s1b")
    s2 = ps_st.tile([1, Tq], F32, name="s2b")
    for e in range(KE):
        xbf = p_tmp.tile([P, Tq], BF16, name="xbf2")
        nc.vector.tensor_copy(xbf[:], xres[e][:])
        x2 = p_tmp.tile([P, Tq], BF16, name="x2bf2")
        nc.scalar.square(x2[:], xres[e][:])
        nc.tensor.matmul(s1[:], ones_bf[:], xbf[:], start=(e == 0), stop=(e == KE - 1))
        nc.tensor.matmul(s2[:], ones_bf[:], x2[:], start=(e == 0), stop=(e == KE - 1))
    mu = p_rows.tile([1, Tq], F32, name="mu_2")
    nc.vector.tensor_scalar_mul(mu[:], s1[:], 1.0 / E)
    ve = p_rows.tile([1, Tq], F32, name="ve_2")
    nc.vector.tensor_scalar(ve[:], s2[:], 1.0 / E, c.eps, OP.mult, OP.add)
    mu2 = p_rows.tile([1, Tq], F32, name="mu2_2")
    nc.vector.tensor_tensor(mu2[:], mu[:], mu[:], OP.mult)
    vee = p_rows.tile([1, Tq], F32, name="vee_2")
    nc.vector.tensor_tensor(vee[:], ve[:], mu2[:], OP.subtract)
    lv = p_rows.tile([1, Tq], F32, name="lv_2")
    nc.scalar.activation(lv[:], vee[:], AF.Ln)
    rstd = p_rows.tile([1, Tq], F32, name="rstd_2")
    nc.scalar.activation(rstd[:], lv[:], AF.Exp, scale=-0.5)
    mub = p_bc2.tile([P, Tq], F32, name="mub2")
    nc.gpsimd.partition_broadcast(mub[:], mu[:])
    rsb = p_bc2.tile([P, Tq], F32, name="rsb2")
    nc.gpsimd.partition_broadcast(rsb[:], rstd[:])
    for e in range(KE):
        eng = nc.vector if e % 2 == 0 else nc.gpsimd
        t1 = p_tmp.tile([P, Tq], F32, name=f"t1b_{e % 2}")
        eng.tensor_tensor(t1[:], xres[e][:], mub[:], OP.subtract)
        t2 = p_tmp.tile([P, Tq], F32, name=f"t2b_{e % 2}")
        eng.tensor_tensor(t2[:], t1[:], rsb[:], OP.mult)
        eng.tensor_scalar(
            xn2[e][:], t2[:],
            gbt["ln2g"][:, e : e + 1], gbt["ln2b"][:, e : e + 1],
            OP.mult, OP.add,
        )
    pp.close("ln2_st", "ln2_bcs", "ln2_rows", "ln2_tmp")

    # ======================================================================
    # Phase 4: MLP (layer 1 streamed with first 6 e-tiles of layer 2)
    # ======================================================================
    EH = min(KE, 6)
    p_h1 = pp.open("h1p", 1, side="right")
    p_out = pp.open("outp", 2)
    ps_h1 = pp.open("h1_ps", 2, "PSUM")
    ps_h2a = pp.open("h2a_ps", 1, "PSUM")

    h1 = [p_h1.tile([P, Tq], BF16, name=f"h1{f}") for f in range(KF)]
    h2a = [ps_h2a.tile([P, Tq], F32, name=f"h2a{e}") for e in range(EH)]
    for f in range(KF):
        ps = ps_h1.tile([P, Tq], F32, name="h1ps")
        for e in range(KE):
            nc.tensor.matmul(
                ps[:], w1t[:, e, f * P : (f + 1) * P], xn2[e][:],
                start=(e == 0), stop=(e == KE - 1),
            )
        nc.scalar.activation(
            h1[f][:], ps[:], AF.Relu, bias=gbt["mb1"][:, f : f + 1], scale=1.0
        )
        for e in range(EH):
            nc.tensor.matmul(
                h2a[e][:], w2t[:, f, e * P : (e + 1) * P], h1[f][:],
                start=(f == 0), stop=(f == KF - 1),
            )
    for e in range(EH):
        of = p_out.tile([P, Tq], F32, name="outf")
        nc.vector.scalar_tensor_tensor(
            of[:], h2a[e][:], gbt["mb2"][:, e : e + 1], xres[e][:], OP.add, OP.add
        )
        nc.sync.dma_start(d["out_t"][e * P : (e + 1) * P, :], of[:])
    pp.close("h2a_ps", "h1_ps")

    if EH < KE:
        ps_h2b = pp.open("h2b_ps", 1, "PSUM")
        h2b = [ps_h2b.tile([P, Tq], F32, name=f"h2b{e}") for e in range(KE - EH)]
        for f in range(KF):
            for i, e in enumerate(range(EH, KE)):
                nc.tensor.matmul(
                    h2b[i][:], w2t[:, f, e * P : (e + 1) * P], h1[f][:],
                    start=(f == 0), stop=(f == KF - 1),
                )
        for i, e in enumerate(range(EH, KE)):
            of = p_out.tile([P, Tq], F32, name="outf")
            nc.vector.scalar_tensor_tensor(
                of[:], h2b[i][:], gbt["mb2"][:, e : e + 1], xres[e][:], OP.add, OP.add
            )
            nc.sync.dma_start(d["out_t"][e * P : (e + 1) * P, :], of[:])

    pp.close_all()


def build_program(c: Cfg = CFG):
    c.check()
    nc = bacc.Bacc(
        "TRN2",
        target_bir_lowering=False,
        debug=False,
        enable_asserts=False,
        num_devices=c.NC,
    )
    d = {}
    d["xt"] = nc.dram_tensor("xt", [c.E, c.T], BF16, kind="ExternalInput").ap()
    d["x_own"] = nc.dram_tensor("x_own", [c.E, c.Tq], F32, kind="ExternalInput").ap()
    d["wq"] = nc.dram_tensor("wq", [P, c.KE, 2 * P], BF16, kind="ExternalInput").ap()
    d["wk"] = nc.dram_tensor("wk", [P, c.KE, 2 * P], BF16, kind="ExternalInput").ap()
    d["wv"] = nc.dram_tensor("wv", [P, c.KE, 2 * P], BF16, kind="ExternalInput").ap()
    d["wo"] = nc.dram_tensor("wo", [P, c.JC, c.E], BF16, kind="ExternalInput").ap()
    d["w1"] = nc.dram_tensor("w1", [P, c.KE, c.F], BF16, kind="ExternalInput").ap()
    d["w2"] = nc.dram_tensor("w2", [P, c.KF, c.E], BF16, kind="ExternalInput").ap()
    for nm, cols in [
        ("ln1g", c.KE), ("ln1b", c.KE), ("ln2g", c.KE), ("ln2b", c.KE),
        ("boc", c.KE), ("mb1", c.KF), ("mb2", c.KE),
    ]:
        d[nm] = nc.dram_tensor(nm, [P, cols], F32, kind="ExternalInput").ap()
    d["mask01"] = nc.dram_tensor(
        "mask01", [P, 4 * c.Tq], BF16, kind="ExternalInput"
    ).ap()
    d["ident"] = nc.dram_tensor("ident", [P, P], BF16, kind="ExternalInput").ap()
    d["sel64"] = nc.dram_tensor("sel64", [4, 4 * 64], BF16, kind="ExternalInput").ap()
    d["out_t"] = nc.dram_tensor("out_t", [c.E, c.Tq], F32, kind="ExternalOutput").ap()

    with tile.TileContext(nc) as tc:
        _emit(tc, c, d)
    nc.compile()
    return nc


# --------------------------------------------------------------------------
# host side
# --------------------------------------------------------------------------
def _pack_lhsT(w, cols_per_block):
    """[R, C] -> [128, R//128, C] lhsT layout (contiguous DMA)."""
    R, C = w.shape
    assert R % P == 0 and C == cols_per_block
    return np.ascontiguousarray(
        w.reshape(R // P, P, C).transpose(1, 0, 2)
    )


def shard_inputs(inputs, c: Cfg = CFG):
    x = np.ascontiguousarray(np.asarray(inputs["x"], np.float32))
    bf = lambda a: np.ascontiguousarray(np.asarray(a, np.float32)).astype(NPBF16)

    chunks = lambda v, k: np.ascontiguousarray(
        np.asarray(v, np.float32).reshape(k, P).T
    )
    com = {
        "w1": _pack_lhsT(bf(inputs["W1"]), c.F),
        "w2": _pack_lhsT(bf(inputs["W2"]), c.E),
        "ln1g": chunks(inputs["ln1_g"], c.KE),
        "ln1b": chunks(inputs["ln1_b"], c.KE),
        "ln2g": chunks(inputs["ln2_g"], c.KE),
        "ln2b": chunks(inputs["ln2_b"], c.KE),
        "boc": chunks(inputs["bo"], c.KE),
        "mb1": chunks(inputs["b1"], c.KF),
        "mb2": chunks(inputs["b2"], c.KE),
    }

    p_idx = np.arange(P)[:, None]
    tq_idx = np.arange(c.Tq)[None, :]
    mask = np.zeros((P, 4 * c.Tq), np.float32)
    for jb in range(4):
        mask[:, jb * c.Tq : (jb + 1) * c.Tq] = np.where(
            tq_idx >= (jb * P + p_idx), 0.0, -1.0e9
        )
    com["mask01"] = mask.astype(NPBF16)
    com["ident"] = np.eye(P, dtype=np.float32).astype(NPBF16)

    sel = np.zeros((4, 4 * 64), np.float32)
    for h in range(4):
        sel[h, h * 64 : (h + 1) * 64] = 1.0
    com["sel64"] = sel.astype(NPBF16)

    Wq, Wk, Wv = (bf(inputs[k]) for k in ("Wq", "Wk", "Wv"))
    Wo = bf(inputs["Wo"])
    maps = []
    for core in range(c.NC):
        b, r = core // c.CPB, core % c.CPB
        hs = slice(r * 2 * P, (r + 1) * 2 * P)  # this core's 256 head-features
        m = dict(com)
        m["xt"] = np.ascontiguousarray(x[b].T).astype(NPBF16)
        m["x_own"] = np.ascontiguousarray(x[b, r * c.Tq : (r + 1) * c.Tq, :].T)
        m["wq"] = _pack_lhsT(Wq[:, hs], 2 * P)
        m["wk"] = _pack_lhsT(Wk[:, hs], 2 * P)
        m["wv"] = _pack_lhsT(Wv[:, hs], 2 * P)
        m["wo"] = _pack_lhsT(np.ascontiguousarray(Wo[hs, :]), c.E)
        maps.append(m)
    return maps


def assemble(results, c: Cfg = CFG):
    out = np.empty((c.B, c.T, c.E), np.float32)
    for core in range(c.NC):
        b, r = core // c.CPB, core % c.CPB
        out[b, r * c.Tq : (r + 1) * c.Tq, :] = results[core]["out_t"].T
    return out


_NC_CACHE = {}


def _get_nc(c: Cfg = CFG):
    if c not in _NC_CACHE:
        _NC_CACHE[c] = build_program(c)
    return _NC_CACHE[c]


LAST_RESULT = None


def kernel(**inputs):
    global LAST_RESULT
    c = CFG
    nc = _get_nc(c)
    maps = shard_inputs(inputs, c)
    res = bass_utils.run_bass_kernel_spmd(nc, maps, core_ids=list(range(c.NC)))
    LAST_RESULT = res
    return assemble(res.results, c)


# revision 22
# speedup vs baseline: 1.3377x; 1.0650x over previous
"""Trainium2 Bass kernel for a pre-norm transformer block (dense_transformer).

Computation (per reference):
    x = x + Attn(LN1(x));  x = x + MLP(LN2(x))
with causal multi-head attention (H=16 heads, D=64) and a 4E ReLU MLP.

Sharding: DP-2 on batch x TP-4 on heads.  Core c = b*4 + r computes
LN1(x_b) over all T tokens, Q/K/V + causal attention for heads 4r..4r+3
only (so K/V projections are not recomputed 4x and score tiles above the
causal diagonal are skipped entirely), then the partial out-projection
for all tokens.  A single bf16 ReduceScatter over each 4-core group sums
the head-group partials and hands core r its 512-token slice, on which it
runs residual + LN2 + the full 4E MLP.

Layouts are feature-major throughout (E on partitions, tokens on the free
axis).  Softmax row sums come free from a ones-column appended to V
(M=65 attnV matmuls).  All weights are host-packed into lhsT layout so
every DMA is contiguous.  Matmuls run in bf16 with fp32 PSUM.
"""

from dataclasses import dataclass

import numpy as np
import ml_dtypes

import concourse.bass as bass  # noqa: F401
import concourse.mybir as mybir
import concourse.tile as tile
from concourse import bacc
from concourse import bass_utils

F32 = mybir.dt.float32
BF16 = mybir.dt.bfloat16
AF = mybir.ActivationFunctionType
OP = mybir.AluOpType
NPBF16 = ml_dtypes.bfloat16

P = 128


@dataclass(frozen=True)
class Cfg:
    B: int = 2
    T: int = 2048
    E: int = 1024
    H: int = 16
    D: int = 64
    NC: int = 8
    eps: float = 1e-5

    @property
    def CPB(self):  # cores per batch (TP group size)
        return self.NC // self.B

    @property
    def Tq(self):  # tokens owned per core (MLP stage)
        return self.T // self.CPB

    @property
    def KE(self):  # E / 128
        return self.E // P

    @property
    def TK(self):  # T / 128 context tiles
        return self.T // P

    @property
    def HPC(self):  # heads per core
        return self.H // self.CPB

    @property
    def JC(self):  # 128-row head-pair blocks per core
        return self.HPC // 2

    @property
    def F(self):
        return 4 * self.E

    @property
    def KF(self):
        return self.F // P

    @property
    def NQC(self):  # query chunks of Tq
        return self.T // self.Tq

    def check(self):
        assert self.D == 64 and self.E == self.H * self.D
        assert self.Tq == 512 and self.HPC == 4 and self.JC == 2
        assert self.T % P == 0 and self.E % P == 0 and self.F % P == 0


CFG = Cfg()


class Pools:
    """Tile pools with explicit open/close (LIFO per side, per space)."""

    def __init__(self, tc, prefix=""):
        self.tc = tc
        self.prefix = prefix
        self.live = {}

    def open(self, key, bufs, space=None, side=None):
        kw = dict(name=self.prefix + key, bufs=bufs)
        if space:
            kw["space"] = space
        if side:
            kw["side"] = side
        cm = self.tc.tile_pool(**kw)
        pool = cm.__enter__()
        self.live[key] = cm
        return pool

    def close(self, *keys):
        for key in keys:
            self.live.pop(key).__exit__(None, None, None)

    def close_all(self):
        for key in reversed(list(self.live)):
            self.close(key)


def _emit(tc, c: Cfg, d):
    nc = tc.nc
    E, T, Tq = c.E, c.T, c.Tq
    KE, TK, KF, JC, NQC, HPC = c.KE, c.TK, c.KF, c.JC, c.NQC, c.HPC
    DV = 65  # V cols per head incl. ones column
    SCL = 1.0 / float(np.sqrt(c.D))

    pp = Pools(tc)

    # ---------------- constants (whole-kernel lifetime) --------------------
    const = pp.open("const", 1)
    ones_bf = const.tile([P, 1], BF16, name="ones_bf")
    nc.vector.memset(ones_bf[:], 1.0)
    ones_f1 = const.tile([1, P], F32, name="ones_f1")
    nc.vector.memset(ones_f1[:], 1.0)
    sel64 = const.tile([4, 4 * 64], BF16, name="sel64")
    nc.sync.dma_start(sel64[:], d["sel64"])
    gbt = {}
    for nm, cols in [
        ("ln1g", KE), ("ln1b", KE), ("ln2g", KE), ("ln2b", KE),
        ("boc", KE), ("mb1", KF), ("mb2", KE),
    ]:
        gbt[nm] = const.tile([P, cols], F32, name=nm + "_t")
        nc.sync.dma_start(gbt[nm][:], d[nm])

    ident = const.tile([P, P], BF16, name="ident")
    nc.sync.dma_start(ident[:], d["ident"])
    wrm = const.tile([P, Tq], BF16, name="wrm")
    nc.vector.memset(wrm[:], 0.0)

    p_band = pp.open("bandp", 1)
    mask01 = p_band.tile([P, 4 * Tq], BF16, name="mask01")
    nc.sync.dma_start(mask01[:], d["mask01"])

    # ---------------- DRAM bounce buffers for the ReduceScatter ------------
    p_dram = pp.open("dram", 1, "DRAM")
    bounce_in = p_dram.tile([NQC * E, Tq], BF16, name="bounce_in")
    bounce_out = p_dram.tile([E, Tq], BF16, name="bounce_out")

    # ---------------- PE warmup (pstate ramp) ------------------------------
    ps_wm = pp.open("warm_ps", 1, "PSUM")
    wmp = ps_wm.tile([1, Tq], F32, name="wmp")
    for _w in range(24):
        nc.tensor.matmul(wmp[:], ones_bf[:], wrm[:], start=True, stop=True)
    pp.close("warm_ps")

    # ======================================================================
    # Phase 0: load x^T (bf16) + LayerNorm1 over all T -> xn (bf16)
    # ======================================================================
    p_kq = pp.open("kqp", 1)
    p_vs = pp.open("vsp", 1)
    qt = [p_kq.tile([P, T], BF16, name=f"qt{j}") for j in range(JC)]
    kt = [p_kq.tile([P, T], BF16, name=f"kt{j}") for j in range(JC)]
    vsb = p_vs.tile([P, TK, HPC, DV], BF16, name="vsb")
    nc.vector.memset(vsb[:, :, :, DV - 1 : DV], 1.0)

    p_xn = pp.open("xnp", 1)
    p_w3 = pp.open("w3p", 1)
    wqt = p_w3.tile([P, KE, 2 * P], BF16, name="wqt")
    wkt = p_w3.tile([P, KE, 2 * P], BF16, name="wkt")
    wvt = p_w3.tile([P, KE, 2 * P], BF16, name="wvt")
    nc.sync.dma_start(wkt[:], d["wk"])
    p_xt = pp.open("xtp", 1)
    xt = [p_xt.tile([P, T], BF16, name=f"xt{e}") for e in range(KE)]
    xn = [p_xn.tile([P, T], BF16, name=f"xn{e}") for e in range(KE)]
    for e in range(KE):
        nc.sync.dma_start(xt[e][:], d["xt"][e * P : (e + 1) * P, :])
    nc.sync.dma_start(wqt[:], d["wq"])
    nc.sync.dma_start(wvt[:], d["wv"])

    p_tmp = pp.open("ln_tmp", 4)
    p_rows = pp.open("ln_rows", 1)
    p_bcs = pp.open("ln_bcs", 2)
    ps_st = pp.open("ln_st", 1, "PSUM")

    for ci in range(NQC):
        cs = slice(ci * Tq, (ci + 1) * Tq)
        s1 = ps_st.tile([1, Tq], F32, name="s1")
        s2 = ps_st.tile([1, Tq], F32, name="s2")
        for e in range(KE):
            x2 = p_tmp.tile([P, Tq], BF16, name="x2bf")
            nc.scalar.square(x2[:], xt[e][:, cs])
            nc.tensor.matmul(s1[:], ones_bf[:], xt[e][:, cs], start=(e == 0), stop=(e == KE - 1))
            nc.tensor.matmul(s2[:], ones_bf[:], x2[:], start=(e == 0), stop=(e == KE - 1))
        mu = p_rows.tile([1, Tq], F32, name="mu")
        nc.vector.tensor_scalar_mul(mu[:], s1[:], 1.0 / E)
        ve = p_rows.tile([1, Tq], F32, name="ve")
        nc.vector.tensor_scalar(ve[:], s2[:], 1.0 / E, c.eps, OP.mult, OP.add)
        mu2 = p_rows.tile([1, Tq], F32, name="mu2")
        nc.vector.tensor_tensor(mu2[:], mu[:], mu[:], OP.mult)
        vee = p_rows.tile([1, Tq], F32, name="vee")
        nc.vector.tensor_tensor(vee[:], ve[:], mu2[:], OP.subtract)
        lv = p_rows.tile([1, Tq], F32, name="lv")
        nc.scalar.activation(lv[:], vee[:], AF.Ln)
        rstd = p_rows.tile([1, Tq], F32, name="rstd")
        nc.scalar.activation(rstd[:], lv[:], AF.Exp, scale=-0.5)

        mub = p_bcs.tile([P, Tq], F32, name="mub")
        nc.gpsimd.partition_broadcast(mub[:], mu[:])
        rsb = p_bcs.tile([P, Tq], F32, name="rsb")
        nc.gpsimd.partition_broadcast(rsb[:], rstd[:])

        for e in range(KE):
            # alternate DVE / gpsimd so the normalize stream isn't one-engine
            eng = nc.vector if e % 2 == 0 else nc.gpsimd
            t1 = p_tmp.tile([P, Tq], F32, name=f"t1_{e % 2}")
            eng.tensor_tensor(t1[:], xt[e][:, cs], mub[:], OP.subtract)
            t2 = p_tmp.tile([P, Tq], F32, name=f"t2_{e % 2}")
            eng.tensor_tensor(t2[:], t1[:], rsb[:], OP.mult)
            eng.tensor_scalar(
                xn[e][:, cs], t2[:],
                gbt["ln1g"][:, e : e + 1], gbt["ln1b"][:, e : e + 1],
                OP.mult, OP.add,
            )
    pp.close("ln_st", "ln_bcs", "ln_rows", "ln_tmp", "xtp")

    # ======================================================================
    # Phase 1: Q/K/V projections for this core's 4 heads
    # ======================================================================
    ps_qkv = pp.open("qkv_ps", 2, "PSUM")

    for j in range(JC):
        for w_t, dst in ((wkt, kt), (wqt, qt)):
            for ci in range(NQC):
                cs = slice(ci * Tq, (ci + 1) * Tq)
                ps = ps_qkv.tile([P, Tq], F32, name="kq_ps")
                for e in range(KE):
                    nc.tensor.matmul(
                        ps[:], w_t[:, e, j * P : (j + 1) * P], xn[e][:, cs],
                        start=(e == 0), stop=(e == KE - 1),
                    )
                nc.scalar.copy(dst[j][:, cs], ps[:])

    for t in range(TK):
        ps = ps_qkv.tile([P, 2 * P], F32, name="v_ps")
        for e in range(KE):
            nc.tensor.matmul(
                ps[:], xn[e][:, t * P : (t + 1) * P], wvt[:, e, :],
                start=(e == 0), stop=(e == KE - 1),
            )
        nc.scalar.copy(
            vsb[:, t, :, 0:64],
            ps[:].rearrange("p (h v) -> p h v", h=HPC),
        )
    pp.close("qkv_ps", "w3p", "xnp")

    # prefetch the out-proj + MLP weights + residual slice while attention runs
    p_wo = pp.open("wop", 1, side="right")
    wot = p_wo.tile([P, JC, E], BF16, name="wot")
    nc.scalar.dma_start(wot[:], d["wo"])
    p_w12 = pp.open("w12p", 1, side="right")
    w1t = p_w12.tile([P, KE, c.F], BF16, name="w1t")
    w2t = p_w12.tile([P, KF, E], BF16, name="w2t")
    nc.scalar.dma_start(w1t[:], d["w1"])
    nc.scalar.dma_start(w2t[:], d["w2"])
    # ======================================================================
    # Phase 2: causal attention for 4 heads (2 pairs), all query chunks
    # ======================================================================
    p_ao = pp.open("aop", 1)
    p_pr = pp.open("probs", 4)
    p_rst = pp.open("rsst", 1)
    p_st2 = pp.open("rstage", 2)
    ps_av = pp.open("av_ps", 1, "PSUM")
    ps_ss = pp.open("ss_ps", 2, "PSUM")

    all_units = [
        (qc, t, p) for qc in range(NQC) for t in range(4 * qc + 4) for p in range(JC)
    ]
    LOOK = 2  # score-lookahead units (PSUM ring self-regulates via WAR)
    avp = {}

    def emit_ss(i):
        qc, t, p = all_units[i]
        jb = t - 4 * qc  # >= 0 on the causal diagonal band
        qs = slice(qc * Tq, (qc + 1) * Tq)
        ssu = ps_ss.tile([P, 2 * Tq], F32, name="ss")
        for s in (0, 1):
            nc.tensor.matmul(
                ssu[:, s * Tq : (s + 1) * Tq],
                kt[p][s * 64 : (s + 1) * 64, t * P : (t + 1) * P],
                qt[p][s * 64 : (s + 1) * 64, qs],
                start=True, stop=(jb < 0),
                tile_position=(s * 64, 0),
                skip_group_check=True,
            )
        if jb >= 0:
            # add -1e9 above the diagonal straight into the score PSUM
            for s in (0, 1):
                nc.tensor.matmul(
                    ssu[:, s * Tq : (s + 1) * Tq], ident[:],
                    mask01[:, jb * Tq : (jb + 1) * Tq],
                    start=False, stop=True,
                    skip_group_check=True,
                )
        return ssu

    def finalize(qc):
        """Softmax denominators -> normalize -> partial out-proj for chunk qc."""
        st = p_rst.tile([P, HPC * Tq], F32, name="rs_st")
        for h in range(HPC):
            nc.vector.tensor_copy(
                st[64:65, h * Tq : (h + 1) * Tq], avp[h][64:65, :]
            )
        rs4 = p_st2.tile([4, Tq], F32, name="rs4", bufs=1)
        nc.sync.dma_start(rs4[:], st[64:65, :])
        rec4 = p_st2.tile([4, Tq], F32, name="rec4", bufs=1)
        nc.vector.reciprocal(rec4[:], rs4[:])
        irs = p_st2.tile([4, Tq], BF16, name="irs", bufs=1)
        nc.vector.tensor_copy(irs[:], rec4[:])

        aop = [p_ao.tile([P, Tq], BF16, name=f"aop{p}") for p in range(JC)]
        for p in range(JC):
            nb = ps_ss.tile([P, 2 * Tq], F32, name="ss")
            for s in (0, 1):
                nc.tensor.matmul(
                    nb[0:64, s * Tq : (s + 1) * Tq],
                    sel64[:, (2 * p + s) * 64 : (2 * p + s + 1) * 64],
                    irs[:],
                    start=True, stop=True,
                    skip_group_check=True,
                )
            nbs = p_st2.tile([64, 2 * Tq], BF16, name="nbs", bufs=1)
            nc.vector.tensor_copy(nbs[:], nb[0:64, :])
            nc.vector.tensor_tensor(
                aop[p][0:64, :], avp[2 * p][0:64, :], nbs[:, 0:Tq], OP.mult
            )
            ost = p_st2.tile([64, Tq], BF16, name="ost")
            nc.vector.tensor_tensor(
                ost[:], avp[2 * p + 1][0:64, :], nbs[:, Tq : 2 * Tq], OP.mult
            )
            nc.sync.dma_start(aop[p][64:128, :], ost[:])

        for e in range(KE):
            po = ps_ss.tile([P, 2 * Tq], F32, name="ss")
            for p in range(JC):
                nc.tensor.matmul(
                    po[:, 0:Tq], wot[:, p, e * P : (e + 1) * P], aop[p][:],
                    start=(p == 0), stop=(p == JC - 1),
                    skip_group_check=True,
                )
            ob = p_st2.tile([P, Tq], BF16, name="ob")
            nc.vector.tensor_copy(ob[:], po[:, 0:Tq])
            nc.sync.dma_start(
                bounce_in[(qc * KE + e) * P : (qc * KE + e + 1) * P, :], ob[:]
            )

    ss_ring = [emit_ss(i) for i in range(min(LOOK, len(all_units)))]
    for i, (qc, t, p) in enumerate(all_units):
        ntile = 4 * qc + 4
        if t == 0 and p == 0:
            for h in range(HPC):
                avp[h] = ps_av.tile([DV, Tq], F32, name=f"avp{h}")
        pr = p_pr.tile([P, 2 * Tq], BF16, name="pr")
        nc.scalar.activation(pr[:], ss_ring[0][:], AF.Exp, scale=SCL)
        ss_ring.pop(0)
        if i + LOOK < len(all_units):
            ss_ring.append(emit_ss(i + LOOK))
        for s in (0, 1):
            h = 2 * p + s
            nc.tensor.matmul(
                avp[h][:],
                vsb[:, t, h, :],
                pr[:, s * Tq : (s + 1) * Tq],
                start=(t == 0), stop=(t == ntile - 1),
                skip_group_check=True,
            )
        if t == ntile - 1 and p == JC - 1:
            finalize(qc)

    pp.close("ss_ps", "av_ps", "rstage", "rsst", "probs", "aop")
    pp.close("vsp", "kqp", "bandp")

    # ======================================================================
    # Phase 3: ReduceScatter partials; residual + bo -> xres; LN2 -> xn2
    # ======================================================================
    p_xo = pp.open("xop", 1, side="right")
    xown = [p_xo.tile([P, Tq], F32, name=f"xown{e}") for e in range(KE)]
    for e in range(KE):
        nc.scalar.dma_start(xown[e][:], d["x_own"][e * P : (e + 1) * P, :])

    nc.gpsimd.collective_compute(
        "ReduceScatter",
        OP.add,
        replica_groups=[[0, 1, 2, 3], [4, 5, 6, 7]],
        ins=[bounce_in.opt()],
        outs=[bounce_out.opt()],
    )

    p_xr = pp.open("xrp", 1)
    p_x2 = pp.open("xn2p", 1)
    p_att = pp.open("attp", 1)
    xres = [p_xr.tile([P, Tq], F32, name=f"xres{e}") for e in range(KE)]
    xn2 = [p_x2.tile([P, Tq], BF16, name=f"xn2{e}") for e in range(KE)]
    att = [p_att.tile([P, Tq], BF16, name=f"att{e}") for e in range(KE)]
    for e in range(KE):
        nc.sync.dma_start(att[e][:], bounce_out[e * P : (e + 1) * P, :])
        nc.vector.scalar_tensor_tensor(
            xres[e][:], att[e][:], gbt["boc"][:, e : e + 1], xown[e][:],
            OP.add, OP.add,
        )
    pp.close("attp")
    pp.close("xop")

    p_tmp = pp.open("ln2_tmp", 3)
    p_rows = pp.open("ln2_rows", 1)
    p_bc2 = pp.open("ln2_bcs", 1)
    ps_st = pp.open("ln2_st", 1, "PSUM")
    s1 = ps_st.tile([1, Tq], F32, name="s1b")
    s2 = ps_st.tile([1, Tq], F32, name="s2b")
    for e in range(KE):
        xbf = p_tmp.tile([P, Tq], BF16, name="xbf2")
        nc.vector.tensor_copy(xbf[:], xres[e][:])
        x2 = p_tmp.tile([P, Tq], BF16, name="x2bf2")
        nc.scalar.square(x2[:], xres[e][:])
        nc.tensor.matmul(s1[:], ones_bf[:], xbf[:], start=(e == 0), stop=(e == KE - 1))
        nc.tensor.matmul(s2[:], ones_bf[:], x2[:], start=(e == 0), stop=(e == KE - 1))
    mu = p_rows.tile([1, Tq], F32, name="mu_2")
    nc.vector.tensor_scalar_mul(mu[:], s1[:], 1.0 / E)
    ve = p_rows.tile([1, Tq], F32, name="ve_2")
    nc.vector.tensor_scalar(ve[:], s2[:], 1.0 / E, c.eps, OP.mult, OP.add)
    mu2 = p_rows.tile([1, Tq], F32, name="mu2_2")
    nc.vector.tensor_tensor(mu2[:], mu[:], mu[:], OP.mult)
    vee = p_rows.tile([1, Tq], F32, name="vee_2")
    nc.vector.tensor_tensor(vee[:], ve[:], mu2[:], OP.subtract)
    lv = p_rows.tile([1, Tq], F32, name="lv_2")
    nc.scalar.activation(lv[:], vee[:], AF.Ln)
    rstd = p_rows.tile([1, Tq], F32, name="rstd_2")
    nc.scalar.activation(rstd[:], lv[:], AF.Exp, scale=-0.5)
    mub = p_bc2.tile([P, Tq], F32, name="mub2")
    nc.gpsimd.partition_broadcast(mub[:], mu[:])
    rsb = p_bc2.tile([P, Tq], F32, name="rsb2")
    nc.gpsimd.partition_broadcast(rsb[:], rstd[:])
    for e in range(KE):
        eng = nc.vector if e % 2 == 0 else nc.gpsimd
        t1 = p_tmp.tile([P, Tq], F32, name=f"t1b_{e % 2}")
        eng.tensor_tensor(t1[:], xres[e][:], mub[:], OP.subtract)
        t2 = p_tmp.tile([P, Tq], F32, name=f"t2b_{e % 2}")
        eng.tensor_tensor(t2[:], t1[:], rsb[:], OP.mult)
        eng.tensor_scalar(
            xn2[e][:], t2[:],
            gbt["ln2g"][:, e : e + 1], gbt["ln2b"][:, e : e + 1],
            OP.mult, OP.add,
        )
    pp.close("ln2_st", "ln2_bcs", "ln2_rows", "ln2_tmp")

    # ======================================================================
    # Phase 4: MLP (layer 1 streamed with first 6 e-tiles of layer 2)
    # ======================================================================
    EH = min(KE, 6)
    p_h1 = pp.open("h1p", 1, side="right")
    p_out = pp.open("outp", 2)
    ps_h1 = pp.open("h1_ps", 2, "PSUM")
    ps_h2a = pp.open("h2a_ps", 1, "PSUM")

    h1 = [p_h1.tile([P, Tq], BF16, name=f"h1{f}") for f in range(KF)]
    h2a = [ps_h2a.tile([P, Tq], F32, name=f"h2a{e}") for e in range(EH)]
    for f in range(KF):
        ps = ps_h1.tile([P, Tq], F32, name="h1ps")
        for e in range(KE):
            nc.tensor.matmul(
                ps[:], w1t[:, e, f * P : (f + 1) * P], xn2[e][:],
                start=(e == 0), stop=(e == KE - 1),
            )
        nc.scalar.activation(
            h1[f][:], ps[:], AF.Relu, bias=gbt["mb1"][:, f : f + 1], scale=1.0
        )
        for e in range(EH):
            nc.tensor.matmul(
                h2a[e][:], w2t[:, f, e * P : (e + 1) * P], h1[f][:],
                start=(f == 0), stop=(f == KF - 1),
            )
    for e in range(EH):
        of = p_out.tile([P, Tq], F32, name="outf")
        nc.vector.scalar_tensor_tensor(
            of[:], h2a[e][:], gbt["mb2"][:, e : e + 1], xres[e][:], OP.add, OP.add
        )
        nc.sync.dma_start(d["out_t"][e * P : (e + 1) * P, :], of[:])
    pp.close("h2a_ps", "h1_ps")

    if EH < KE:
        ps_h2b = pp.open("h2b_ps", 1, "PSUM")
        h2b = [ps_h2b.tile([P, Tq], F32, name=f"h2b{e}") for e in range(KE - EH)]
        for f in range(KF):
            for i, e in enumerate(range(EH, KE)):
                nc.tensor.matmul(
                    h2b[i][:], w2t[:, f, e * P : (e + 1) * P], h1[f][:],
                    start=(f == 0), stop=(f == KF - 1),
                )
        for i, e in enumerate(range(EH, KE)):
            of = p_out.tile([P, Tq], F32, name="outf")
            nc.vector.scalar_tensor_tensor(
                of[:], h2b[i][:], gbt["mb2"][:, e : e + 1], xres[e][:], OP.add, OP.add
            )
            nc.sync.dma_start(d["out_t"][e * P : (e + 1) * P, :], of[:])

    pp.close_all()


def build_program(c: Cfg = CFG):
    c.check()
    nc = bacc.Bacc(
        "TRN2",
        target_bir_lowering=False,
        debug=False,
        enable_asserts=False,
        num_devices=c.NC,
    )
    d = {}
    d["xt"] = nc.dram_tensor("xt", [c.E, c.T], BF16, kind="ExternalInput").ap()
    d["x_own"] = nc.dram_tensor("x_own", [c.E, c.Tq], F32, kind="ExternalInput").ap()
    d["wq"] = nc.dram_tensor("wq", [P, c.KE, 2 * P], BF16, kind="ExternalInput").ap()
    d["wk"] = nc.dram_tensor("wk", [P, c.KE, 2 * P], BF16, kind="ExternalInput").ap()
    d["wv"] = nc.dram_tensor("wv", [P, c.KE, 2 * P], BF16, kind="ExternalInput").ap()
    d["wo"] = nc.dram_tensor("wo", [P, c.JC, c.E], BF16, kind="ExternalInput").ap()
    d["w1"] = nc.dram_tensor("w1", [P, c.KE, c.F], BF16, kind="ExternalInput").ap()
    d["w2"] = nc.dram_tensor("w2", [P, c.KF, c.E], BF16, kind="ExternalInput").ap()
    for nm, cols in [
        ("ln1g", c.KE), ("ln1b", c.KE), ("ln2g", c.KE), ("ln2b", c.KE),
        ("boc", c.KE), ("mb1", c.KF), ("mb2", c.KE),
    ]:
        d[nm] = nc.dram_tensor(nm, [P, cols], F32, kind="ExternalInput").ap()
    d["mask01"] = nc.dram_tensor(
        "mask01", [P, 4 * c.Tq], BF16, kind="ExternalInput"
    ).ap()
    d["ident"] = nc.dram_tensor("ident", [P, P], BF16, kind="ExternalInput").ap()
    d["sel64"] = nc.dram_tensor("sel64", [4, 4 * 64], BF16, kind="ExternalInput").ap()
    d["out_t"] = nc.dram_tensor("out_t", [c.E, c.Tq], F32, kind="ExternalOutput").ap()

    with tile.TileContext(nc) as tc:
        _emit(tc, c, d)
    nc.compile()
    return nc


# --------------------------------------------------------------------------
# host side
# --------------------------------------------------------------------------
def _pack_lhsT(w, cols_per_block):
    """[R, C] -> [128, R//128, C] lhsT layout (contiguous DMA)."""
    R, C = w.shape
    assert R % P == 0 and C == cols_per_block
    return np.ascontiguousarray(
        w.reshape(R // P, P, C).transpose(1, 0, 2)
    )


def shard_inputs(inputs, c: Cfg = CFG):
    x = np.ascontiguousarray(np.asarray(inputs["x"], np.float32))
    bf = lambda a: np.ascontiguousarray(np.asarray(a, np.float32)).astype(NPBF16)

    chunks = lambda v, k: np.ascontiguousarray(
        np.asarray(v, np.float32).reshape(k, P).T
    )
    com = {
        "w1": _pack_lhsT(bf(inputs["W1"]), c.F),
        "w2": _pack_lhsT(bf(inputs["W2"]), c.E),
        "ln1g": chunks(inputs["ln1_g"], c.KE),
        "ln1b": chunks(inputs["ln1_b"], c.KE),
        "ln2g": chunks(inputs["ln2_g"], c.KE),
        "ln2b": chunks(inputs["ln2_b"], c.KE),
        "boc": chunks(inputs["bo"], c.KE),
        "mb1": chunks(inputs["b1"], c.KF),
        "mb2": chunks(inputs["b2"], c.KE),
    }

    p_idx = np.arange(P)[:, None]
    tq_idx = np.arange(c.Tq)[None, :]
    mask = np.zeros((P, 4 * c.Tq), np.float32)
    for jb in range(4):
        mask[:, jb * c.Tq : (jb + 1) * c.Tq] = np.where(
            tq_idx >= (jb * P + p_idx), 0.0, -1.0e9
        )
    com["mask01"] = mask.astype(NPBF16)
    com["ident"] = np.eye(P, dtype=np.float32).astype(NPBF16)

    sel = np.zeros((4, 4 * 64), np.float32)
    for h in range(4):
        sel[h, h * 64 : (h + 1) * 64] = 1.0
    com["sel64"] = sel.astype(NPBF16)

    Wq, Wk, Wv = (bf(inputs[k]) for k in ("Wq", "Wk", "Wv"))
    Wo = bf(inputs["Wo"])
    maps = []
    for core in range(c.NC):
        b, r = core // c.CPB, core % c.CPB
        hs = slice(r * 2 * P, (r + 1) * 2 * P)  # this core's 256 head-features
        m = dict(com)
        m["xt"] = np.ascontiguousarray(x[b].T).astype(NPBF16)
        m["x_own"] = np.ascontiguousarray(x[b, r * c.Tq : (r + 1) * c.Tq, :].T)
        m["wq"] = _pack_lhsT(Wq[:, hs], 2 * P)
        m["wk"] = _pack_lhsT(Wk[:, hs], 2 * P)
        m["wv"] = _pack_lhsT(Wv[:, hs], 2 * P)
        m["wo"] = _pack_lhsT(np.ascontiguousarray(Wo[hs, :]), c.E)
        maps.append(m)
    return maps


def assemble(results, c: Cfg = CFG):
    out = np.empty((c.B, c.T, c.E), np.float32)
    for core in range(c.NC):
        b, r = core // c.CPB, core % c.CPB
        out[b, r * c.Tq : (r + 1) * c.Tq, :] = results[core]["out_t"].T
    return out


_NC_CACHE = {}


def _get_nc(c: Cfg = CFG):
    if c not in _NC_CACHE:
        _NC_CACHE[c] = build_program(c)
    return _NC_CACHE[c]


LAST_RESULT = None


def kernel(**inputs):
    global LAST_RESULT
    c = CFG
    nc = _get_nc(c)
    maps = shard_inputs(inputs, c)
    res = bass_utils.run_bass_kernel_spmd(nc, maps, core_ids=list(range(c.NC)))
    LAST_RESULT = res
    return assemble(res.results, c)
